# revision 36
# baseline (speedup 1.0000x reference)
"""Multi-head attention with RoPE on 8 Trainium2 NeuronCores (v3).

Problem: x[2,2048,1024] -> MHA(16 heads, hd=64, NeoX RoPE, non-causal) -> out.

Sharding: tensor-parallel over heads. Each core owns 2 heads. All input
layout work (x^T, bf16 casts, doubled cos/sin tables, weight swizzles,
per-core weight column slices) happens host-side in make_in_maps; the
device kernel is pure compute:

  - q^T,k^T (RoPE'd via a permutation matmul) and v^T projections from the
    pre-transposed x^T, full sequence per core,
  - flash-style attention with *transposed* scores [s_k, s_q]; the softmax
    denominator comes from a fused ones-column in V (constant bias inside
    the exp keeps fp32 range safe),
  - AllToAll redistributes unnormalized numerator + sigma rows, split in
    three (batch 0 | batch 1 first half | batch 1 second half) so only the
    last small collective is exposed; Wo matmuls fill its latency,
  - consumer-side 1/sigma via one reciprocal + selector-matmul broadcast,
  - local Wo matmul produces [256 b0 | 128+128 b1] token rows per core.

All matmuls run in bf16 (fp32 PSUM accumulation); rel-err tolerance 2e-2.
"""

import sys

sys.path.insert(0, "/opt/trn_rl_repo")

import numpy as np  # noqa: E402
import ml_dtypes  # noqa: E402

import concourse.bass as bass  # noqa: E402
import concourse.mybir as mybir  # noqa: E402
import concourse.tile as tile  # noqa: E402
from concourse.bass_utils import run_bass_kernel_spmd  # noqa: E402

N_CORES = 8
D = 1024
H = 16
HD = 64
HL = H // N_CORES  # local heads per core
DL = HL * HD  # 128 local attn dims
EXP_SCALE = 0.125  # 1/sqrt(hd)
EXP_BIAS = -24.0  # exp(s/8 - 24): cancels in softmax, keeps fp32 range safe
GMAX = 2  # score-psum kt-tiles per exp instruction

F32 = mybir.dt.float32
BF16 = mybir.dt.bfloat16
BF16_NP = ml_dtypes.bfloat16


def _kt_groups(kt):
    groups = []
    k0 = 0
    while k0 < kt:
        g = min(GMAX, kt - k0)
        if (kt - k0) % GMAX == 1 and GMAX > 1:
            g = min(GMAX - 1, kt - k0)
        groups.append((k0, g))
        k0 += g
    return groups


def _perm_matrix():
    """lhsT for the rotate_half matmul: qrot^T = lhsT.T @ q^T."""
    mt = np.zeros((DL, DL), dtype=np.float32)
    for o in (0, HD):
        for r in range(HD // 2):
            mt[o + r, o + r + HD // 2] = -1.0
            mt[o + r + HD // 2, o + r] = 1.0
    return np.ascontiguousarray(mt.T)


def split_excess_waits(nc, max_waits=1):
    """This container's walrus rejects >1 semaphore wait per instruction;
    split excess waits onto NoOp carriers on the same engine."""
    for bb in nc.m.functions[0].blocks:
        insts = bb.instructions
        idx = 0
        while idx < len(insts):
            ins = insts[idx]
            si = ins.sync_info
            if si is not None and si.on_wait and len(si.on_wait) > max_waits:
                ow = list(si.on_wait)
                si.on_wait = ow[-max_waits:]
                extra = ow[:-max_waits]
                k = 0
                while extra:
                    chunk, extra = extra[:max_waits], extra[max_waits:]
                    c = mybir.InstNoOp(name=f"{ins.name}-ws{k}", ins=[], outs=[])
                    c.engine = ins.engine
                    c.sync_info = mybir.SyncInfo(on_wait=chunk, on_update=[])
                    nc.register_instruction(c)
                    insts.insert(idx, c)
                    idx += 1
                    k += 1
            idx += 1


def build_nc(b=2, s=2048, chunk=512, pt_bufs=12, debug=False):
    kt = s // 128
    nch = s // chunk
    dt8 = D // 128
    shard_half = s // N_CORES  # 256 tokens per core per batch
    groups = _kt_groups(kt)

    nc = bass.Bass()
    # all layout prep is host-side; everything below is bf16 device-ready
    xtp = nc.declare_dram_parameter("xt", [128, b * dt8, s], BF16, isOutput=False)
    csp = nc.declare_dram_parameter("csn", [128, s], BF16, isOutput=False)
    snp = nc.declare_dram_parameter("snn", [128, s], BF16, isOutput=False)
    wqp = nc.declare_dram_parameter("wq", [128, dt8, DL], BF16, isOutput=False)
    wkp = nc.declare_dram_parameter("wk", [128, dt8, DL], BF16, isOutput=False)
    wvp = nc.declare_dram_parameter("wv", [128, dt8, DL], BF16, isOutput=False)
    wop = nc.declare_dram_parameter("wo", [128, dt8, D], BF16, isOutput=False)
    selp = nc.declare_dram_parameter("sel", [H, N_CORES, 128], BF16, isOutput=False)
    mpp = nc.declare_dram_parameter("mperm", [DL, DL], BF16, isOutput=False)
    idp = nc.declare_dram_parameter("ident", [128, 128], BF16, isOutput=False)
    out = nc.declare_dram_parameter("out", [b * shard_half, D], F32, isOutput=True)
    if debug:
        dbg_q = nc.declare_dram_parameter("dbg_q", [b, DL, s], F32, isOutput=True)
        dbg_k = nc.declare_dram_parameter("dbg_k", [b, DL, s], F32, isOutput=True)
        dbg_v = nc.declare_dram_parameter("dbg_v", [b, DL, s], F32, isOutput=True)
        dbg_att = nc.declare_dram_parameter("dbg_att", [b, DL, s], F32, isOutput=True)

    with tile.TileContext(nc) as tc:
        with (
            tc.tile_pool(name="dram", bufs=1, space="DRAM") as dram,
            tc.tile_pool(name="const", bufs=1) as cpool,
            tc.tile_pool(name="xt", bufs=2) as xtpool,
            tc.tile_pool(name="qkv", bufs=2) as qkvpool,
            tc.tile_pool(name="rope", bufs=2) as ropepool,
            tc.tile_pool(name="pt", bufs=pt_bufs) as ptpool,
            tc.tile_pool(name="att", bufs=2) as attpool,
            tc.tile_pool(name="nrm", bufs=1) as nrmpool,
            tc.tile_pool(name="recv", bufs=1) as rcvpool,
            tc.tile_pool(name="outp", bufs=1) as outpool,
            # PSUM: 8 banks. psA = scores (2 tags x 2 banks; projections and
            # Wo borrow). psB = 2 PV banks. psC = 2 banks for v-transposes /
            # rot / bc broadcasts.
            tc.tile_pool(name="psA", bufs=1, space="PSUM") as psA,
            tc.tile_pool(name="psB", bufs=2, space="PSUM") as psB,
            tc.tile_pool(name="psC", bufs=2, space="PSUM") as psC,
        ):
            # ---------- constants (direct bf16 loads, no staging) ----------
            id_sb = cpool.tile([128, 128], BF16, tag="ident")
            nc.sync.dma_start(id_sb[:], idp[:])
            mp_sb = cpool.tile([DL, DL], BF16, tag="mperm")
            nc.sync.dma_start(mp_sb[:], mpp[:])

            # x^T for both batches (one big DMA each; batch 1's overlaps
            # batch-0 compute)
            xt0 = xtpool.tile([128, dt8, s], BF16, tag="xt", name="xt0")
            nc.sync.dma_start(xt0[:], xtp[:, 0:dt8, :])

            wq_sb = cpool.tile([128, dt8, DL], BF16, tag="wq")
            nc.sync.dma_start(wq_sb[:], wqp[:])
            wk_sb = cpool.tile([128, dt8, DL], BF16, tag="wk")
            nc.sync.dma_start(wk_sb[:], wkp[:])
            wv_sb = cpool.tile([128, dt8, DL], BF16, tag="wv")
            nc.sync.dma_start(wv_sb[:], wvp[:])
            cs128 = cpool.tile([128, s], BF16, tag="cs")
            nc.sync.dma_start(cs128[:], csp[:])
            sn128 = cpool.tile([128, s], BF16, tag="sn")
            nc.sync.dma_start(sn128[:], snp[:])
            sel_sb = cpool.tile([H, N_CORES, 128], BF16, tag="sel")
            nc.sync.dma_start(sel_sb[:], selp[:])

            xt1 = xtpool.tile([128, dt8, s], BF16, tag="xt", name="xt1")
            nc.gpsimd.dma_start(xt1[:], xtp[:, dt8 : 2 * dt8, :])

            biasc = cpool.tile([128, 1], F32, tag="biasc")
            nc.vector.memset(biasc[:], EXP_BIAS)

            wo_sb = cpool.tile([128, dt8, D], BF16, tag="wo")

            # ---------- pipeline pieces ----------
            def emit_proj(wsb, dst, ch, xt_sb, rope):
                cols = slice(ch * chunk, (ch + 1) * chunk)
                ps = psA.tile([128, chunk], F32, tag=f"sc{ch % 2}")
                for dt in range(dt8):
                    nc.tensor.matmul(
                        ps[:],
                        wsb[:, dt, :],
                        xt_sb[:, dt, cols],
                        start=(dt == 0),
                        stop=(dt == dt8 - 1),
                    )
                if not rope:
                    nc.vector.tensor_copy(dst[:, cols], ps[:])
                    return
                tsb = ropepool.tile([128, chunk], BF16, tag="tsb")
                nc.scalar.copy(tsb[:], ps[:])
                rps = psC.tile([128, chunk], F32, tag="tp")
                nc.tensor.matmul(rps[:], mp_sb[:], tsb[:], start=True, stop=True)
                m1 = ropepool.tile([128, chunk], BF16, tag="m1")
                nc.vector.tensor_tensor(
                    m1[:], tsb[:], cs128[:, cols], mybir.AluOpType.mult
                )
                m2 = ropepool.tile([128, chunk], BF16, tag="m2")
                nc.vector.tensor_tensor(
                    m2[:], rps[:], sn128[:, cols], mybir.AluOpType.mult
                )
                nc.vector.tensor_tensor(
                    dst[:, cols], m1[:], m2[:], mybir.AluOpType.add
                )

            def emit_vt_group(ch, vt_sb, v_sb):
                vps = psC.tile([128, 4, 128], BF16, tag="tp")
                for j in range(4):
                    ktt = ch * 4 + j
                    nc.tensor.transpose(
                        vps[:, j, :],
                        vt_sb[:, ktt * 128 : (ktt + 1) * 128],
                        id_sb[:],
                    )
                nc.vector.tensor_copy(
                    v_sb[:, ch * 4 : (ch + 1) * 4, :, 0:HD],
                    vps[:].rearrange("p t (h d) -> p t h d", h=HL),
                )

            def emit_attn_chunk(bi, ch, q_rope, k_rope, v_sb, aohs):
                cols = slice(ch * chunk, (ch + 1) * chunk)
                pts = {}
                for gi, (k0, glen) in enumerate(groups):
                    for h in range(HL):
                        rows = slice(h * HD, (h + 1) * HD)
                        sg = psA.tile([128, GMAX, chunk], F32, tag=f"sc{h}")
                        for j in range(glen):
                            ktt = k0 + j
                            nc.tensor.matmul(
                                sg[:, j, :],
                                k_rope[rows, ktt * 128 : (ktt + 1) * 128],
                                q_rope[rows, cols],
                                start=True,
                                stop=True,
                            )
                        pt = ptpool.tile([128, GMAX, chunk], BF16, tag="pt")
                        nc.scalar.activation(
                            pt[:, :glen, :],
                            sg[:, :glen, :],
                            mybir.ActivationFunctionType.Exp,
                            bias=biasc[:],
                            scale=EXP_SCALE,
                        )
                        pts[(gi, h)] = pt
                for h in range(HL):
                    pv = psB.tile([HD + 1, chunk], F32, tag="pv")
                    for gi, (k0, glen) in enumerate(groups):
                        pt = pts[(gi, h)]
                        for j in range(glen):
                            ktt = k0 + j
                            nc.tensor.matmul(
                                pv[:],
                                v_sb[:, ktt, h, :],
                                pt[:, j, :],
                                start=(ktt == 0),
                                stop=(ktt == kt - 1),
                            )
                    # unnormalized numerator + sigma row; 1/sigma applied
                    # once, consumer-side after the A2A
                    nc.vector.tensor_copy(aohs[h][:, cols], pv[:])

            # ---------- batch-0 QKV ----------
            q0 = qkvpool.tile([DL, s], BF16, tag="q_rope", bufs=1)
            k0_ = qkvpool.tile([DL, s], BF16, tag="k_rope")
            vt0 = qkvpool.tile([DL, s], BF16, tag="vt", bufs=1)
            v0 = qkvpool.tile([128, kt, HL, HD + 1], BF16, tag="v_sb")
            nc.vector.memset(v0[:, :, :, HD : HD + 1], 1.0)
            for ch in range(nch):
                emit_proj(wk_sb, k0_, ch, xt0, rope=True)
                emit_proj(wv_sb, vt0, ch, xt0, rope=False)
                emit_vt_group(ch, vt0, v0)
            for ch in range(nch):
                emit_proj(wq_sb, q0, ch, xt0, rope=True)

            # Wo load: off the critical path, overlaps batch-0 attention
            nc.sync.dma_start(wo_sb[:], wop[:])

            # ---------- batch-0 attention, batch-1 kv interleaved ----------
            ao0 = [
                attpool.tile([HD + 1, s], BF16, tag=f"aoh{h}", name=f"ao0_{h}")
                for h in range(HL)
            ]
            q1 = qkvpool.tile([DL, s], BF16, tag="q_rope", bufs=1)
            k1 = qkvpool.tile([DL, s], BF16, tag="k_rope")
            vt1 = qkvpool.tile([DL, s], BF16, tag="vt", bufs=1)
            v1 = qkvpool.tile([128, kt, HL, HD + 1], BF16, tag="v_sb")
            for ch in range(nch):
                emit_attn_chunk(0, ch, q0, k0_, v0, ao0)
                if ch == 0:
                    nc.vector.memset(v1[:, :, :, HD : HD + 1], 1.0)
                emit_proj(wk_sb, k1, ch, xt1, rope=True)
                emit_proj(wv_sb, vt1, ch, xt1, rope=False)
                emit_vt_group(ch, vt1, v1)

            # ---------- A2A / Wo ----------
            def emit_a2a(aohs, col0, w, tag):
                """AllToAll of tokens [col0, col0 + 8*w) (w per peer).
                rows 0..127: attn dims (h0, h1); rows 128..129: sigma."""
                a2a_in = dram.tile(
                    [N_CORES, DL + HL, w], BF16, tag=f"a2a_in{tag}",
                    name=f"a2a_in{tag}",
                )
                a2a_out = dram.tile(
                    [N_CORES, DL + HL, w], BF16, tag=f"a2a_out{tag}",
                    name=f"a2a_out{tag}",
                )
                for h in range(HL):
                    nc.sync.dma_start(
                        a2a_in[:, h * HD : (h + 1) * HD, :].rearrange(
                            "j r c -> r j c"
                        ),
                        aohs[h][0:HD, col0 : col0 + N_CORES * w].rearrange(
                            "r (j c) -> r j c", j=N_CORES
                        ),
                    )
                    nc.sync.dma_start(
                        a2a_in[:, DL + h : DL + h + 1, :].rearrange("j r c -> r j c"),
                        aohs[h][HD : HD + 1, col0 : col0 + N_CORES * w].rearrange(
                            "r (j c) -> r j c", j=N_CORES
                        ),
                    )
                nc.gpsimd.collective_compute(
                    "AllToAll",
                    mybir.AluOpType.bypass,
                    replica_groups=[list(range(N_CORES))],
                    ins=[a2a_in.opt()],
                    outs=[a2a_out.opt()],
                )
                return a2a_out

            def emit_wo(a2a_out, w, out_row0, tg):
                recv = rcvpool.tile(
                    [DL, N_CORES, w], BF16, tag=f"recv{tg}", name="recv"
                )
                nc.sync.dma_start(
                    recv[:], a2a_out[:, 0:DL, :].rearrange("j r c -> r j c")
                )
                # sigr row h*8+i = sigma of source core i's local head h
                sigr = rcvpool.tile([H, w], BF16, tag=f"sigr{tg}", name="sigr")
                for h in range(HL):
                    nc.sync.dma_start(
                        sigr[h * N_CORES : (h + 1) * N_CORES, :],
                        a2a_out[:, DL + h, :],
                    )
                sigf = nrmpool.tile([H, w], F32, tag=f"sigf{tg}", name="sigf")
                nc.vector.tensor_copy(sigf[:], sigr[:])
                rcpf = nrmpool.tile([H, w], F32, tag=f"rcpf{tg}", name="rcpf")
                nc.vector.reciprocal(rcpf[:], sigf[:])
                rcpb = nrmpool.tile([H, w], BF16, tag=f"rcpb{tg}", name="rcpb")
                nc.vector.tensor_copy(rcpb[:], rcpf[:])
                bcs = rcvpool.tile(
                    [DL, N_CORES, w], BF16, tag=f"bcs{tg}", name="bcs"
                )
                for i2 in range(N_CORES // 2):
                    bcp = psC.tile([128, 2, w], F32, tag="tp", name="bcp")
                    for k in range(2):
                        i = 2 * i2 + k
                        nc.tensor.matmul(
                            bcp[:, k, :],
                            sel_sb[:, i, :],
                            rcpb[:],
                            start=True,
                            stop=True,
                        )
                    nc.vector.tensor_copy(bcs[:, 2 * i2 : 2 * i2 + 2, :], bcp[:])
                nc.vector.tensor_tensor(
                    recv[:], recv[:], bcs[:], mybir.AluOpType.mult
                )
                for j in range(w // 128):
                    osb = outpool.tile([128, D], F32, tag="osb", name="osb")
                    for nco in range(D // chunk):
                        wps = psA.tile(
                            [128, chunk], F32, tag=f"sc{(j + nco) % 2}", name="wps"
                        )
                        for i in range(N_CORES):
                            nc.tensor.matmul(
                                wps[:],
                                recv[:, i, j * 128 : (j + 1) * 128],
                                wo_sb[:, i, nco * chunk : (nco + 1) * chunk],
                                start=(i == 0),
                                stop=(i == N_CORES - 1),
                            )
                        nc.scalar.copy(osb[:, nco * chunk : (nco + 1) * chunk], wps[:])
                    nc.sync.dma_start(
                        out[out_row0 + j * 128 : out_row0 + (j + 1) * 128, :],
                        osb[:],
                    )

            if debug:
                for name, tl in (("dbg_q", q0), ("dbg_k", k0_), ("dbg_v", vt0)):
                    for cch in range(nch):
                        df = outpool.tile([DL, chunk], F32, tag="dbgf")
                        nc.vector.tensor_copy(
                            df[:], tl[:, cch * chunk : (cch + 1) * chunk]
                        )
                        nc.sync.dma_start(
                            {"dbg_q": dbg_q, "dbg_k": dbg_k, "dbg_v": dbg_v}[name][0][
                                :, cch * chunk : (cch + 1) * chunk
                            ],
                            df[:],
                        )

            a2a_out0 = emit_a2a(ao0, 0, shard_half, "b0")

            # ---------- batch-1 q + attention ----------
            ao1 = [
                attpool.tile([HD + 1, s], BF16, tag=f"aoh{h}", name=f"ao1_{h}")
                for h in range(HL)
            ]
            a2a_out1a = None
            # q projections run 2 chunks ahead of attention so the rope
            # chain latency never gates the score matmuls
            emit_proj(wq_sb, q1, 0, xt1, rope=True)
            emit_proj(wq_sb, q1, 1, xt1, rope=True)
            for ch in range(nch):
                if ch + 2 < nch:
                    emit_proj(wq_sb, q1, ch + 2, xt1, rope=True)
                emit_attn_chunk(1, ch, q1, k1, v1, ao1)
                if ch == 1:
                    # first half of batch 1 ships now; hides under ch2-3
                    a2a_out1a = emit_a2a(ao1, 0, shard_half // 2, "b1a")

            if debug:
                for bi, ao in ((0, ao0), (1, ao1)):
                    for h in range(HL):
                        for cch in range(nch):
                            df = outpool.tile([HD, chunk], F32, tag="dbgf2")
                            nc.vector.tensor_copy(
                                df[:], ao[h][0:HD, cch * chunk : (cch + 1) * chunk]
                            )
                            nc.sync.dma_start(
                                dbg_att[
                                    bi,
                                    h * HD : (h + 1) * HD,
                                    cch * chunk : (cch + 1) * chunk,
                                ],
                                df[:],
                            )

            # last collective first, then ALL Wo work fills its latency
            a2a_out1b = emit_a2a(ao1, s // 2, shard_half // 2, "b1b")
            emit_wo(a2a_out0, shard_half, 0, "b0")
            emit_wo(a2a_out1a, shard_half // 2, shard_half, "b1a")
            emit_wo(a2a_out1b, shard_half // 2, shard_half + shard_half // 2, "b1b")

    split_excess_waits(nc)
    return nc


def _host_prep(x, cos, sin, b, s):
    """Device-ready layouts shared across cores."""
    bt = b * s
    # x^T in the projection's contraction layout: [128, b*dt8, s]
    xt = np.ascontiguousarray(x.reshape(bt, D).T.astype(BF16_NP))  # [D, b*s]
    xt = (
        xt.reshape(D // 128, 128, b, s)
        .transpose(1, 2, 0, 3)
        .reshape(128, b * (D // 128), s)
    )
    xt = np.ascontiguousarray(xt)
    # doubled, transposed rope tables [128, s]: row p = table[t, p % 32]
    csn = np.ascontiguousarray(np.tile(cos.T, (4, 1)).astype(BF16_NP))
    snn = np.ascontiguousarray(np.tile(sin.T, (4, 1)).astype(BF16_NP))
    # selector for the consumer-side 1/sigma broadcast (sigr is h-major)
    selm = np.zeros((H, N_CORES, 128), dtype=np.float32)
    for i in range(N_CORES):
        for p in range(128):
            selm[(p // HD) * N_CORES + i, i, p] = 1.0
    selb = np.ascontiguousarray(selm.astype(BF16_NP))
    mperm = np.ascontiguousarray(_perm_matrix().astype(BF16_NP))
    ident = np.ascontiguousarray(np.eye(128, dtype=np.float32).astype(BF16_NP))
    return xt, csn, snn, selb, mperm, ident


def _swz(w):  # [D, M] -> [128, dt8, M] bf16
    m = w.shape[1]
    return np.ascontiguousarray(
        np.asarray(w, dtype=np.float32)
        .reshape(D // 128, 128, m)
        .transpose(1, 0, 2)
        .astype(BF16_NP)
    )


def make_in_maps(x, cos, sin, Wq, Wk, Wv, Wo, b, s):
    xt, csn, snn, selb, mperm, ident = _host_prep(
        np.asarray(x, dtype=np.float32),
        np.asarray(cos, dtype=np.float32),
        np.asarray(sin, dtype=np.float32),
        b, s,
    )
    wo_s = _swz(Wo)
    in_maps = []
    for c in range(N_CORES):
        cs = slice(c * DL, (c + 1) * DL)
        in_maps.append(
            {
                "xt": xt,
                "csn": csn,
                "snn": snn,
                "wq": _swz(Wq[:, cs]),
                "wk": _swz(Wk[:, cs]),
                "wv": _swz(Wv[:, cs]),
                "wo": wo_s,
                "sel": selb,
                "mperm": mperm,
                "ident": ident,
            }
        )
    return in_maps


_NC_CACHE = {}


def run(x, cos, sin, Wq, Wk, Wv, Wo, trace=False, chunk=512, pt_bufs=12,
        debug=False):
    b, s, _ = x.shape
    key = (b, s, chunk, pt_bufs, debug)
    if key not in _NC_CACHE:
        _NC_CACHE[key] = build_nc(
            b=b, s=s, chunk=chunk, pt_bufs=pt_bufs, debug=debug
        )
    nc = _NC_CACHE[key]
    in_maps = make_in_maps(x, cos, sin, Wq, Wk, Wv, Wo, b, s)
    res = run_bass_kernel_spmd(nc, in_maps, list(range(N_CORES)), trace=trace)
    sh = s // N_CORES  # 256
    hh = sh // 2  # 128
    b0 = np.concatenate(
        [res.results[c]["out"][0:sh] for c in range(N_CORES)], axis=0
    )
    b1a = np.concatenate(
        [res.results[c]["out"][sh : sh + hh] for c in range(N_CORES)], axis=0
    )
    b1b = np.concatenate(
        [res.results[c]["out"][sh + hh : 2 * sh] for c in range(N_CORES)], axis=0
    )
    full = np.stack([b0, np.concatenate([b1a, b1b], axis=0)], axis=0)
    return full.reshape(b, s, D), res


def kernel(x, cos, sin, Wq, Wk, Wv, Wo):
    out, _ = run(
        np.asarray(x), np.asarray(cos), np.asarray(sin),
        np.asarray(Wq), np.asarray(Wk), np.asarray(Wv), np.asarray(Wo),
    )
    return out.astype(np.float32)


# revision 37
# speedup vs baseline: 1.0485x; 1.0485x over previous
"""Multi-head attention with RoPE on 8 Trainium2 NeuronCores (v3).

Problem: x[2,2048,1024] -> MHA(16 heads, hd=64, NeoX RoPE, non-causal) -> out.

Sharding: tensor-parallel over heads. Each core owns 2 heads. All input
layout work (x^T, bf16 casts, doubled cos/sin tables, weight swizzles,
per-core weight column slices) happens host-side in make_in_maps; the
device kernel is pure compute:

  - q^T,k^T (RoPE'd via a permutation matmul) and v^T projections from the
    pre-transposed x^T, full sequence per core,
  - flash-style attention with *transposed* scores [s_k, s_q]; the softmax
    denominator comes from a fused ones-column in V (constant bias inside
    the exp keeps fp32 range safe),
  - AllToAll redistributes unnormalized numerator + sigma rows, split in
    three (batch 0 | batch 1 first half | batch 1 second half) so only the
    last small collective is exposed; Wo matmuls fill its latency,
  - consumer-side 1/sigma via one reciprocal + selector-matmul broadcast,
  - local Wo matmul produces [256 b0 | 128+128 b1] token rows per core.

All matmuls run in bf16 (fp32 PSUM accumulation); rel-err tolerance 2e-2.
"""

import sys

sys.path.insert(0, "/opt/trn_rl_repo")

import numpy as np  # noqa: E402
import ml_dtypes  # noqa: E402

import concourse.bass as bass  # noqa: E402
import concourse.mybir as mybir  # noqa: E402
import concourse.tile as tile  # noqa: E402
from concourse.bass_utils import run_bass_kernel_spmd  # noqa: E402

N_CORES = 8
D = 1024
H = 16
HD = 64
HL = H // N_CORES  # local heads per core
DL = HL * HD  # 128 local attn dims
EXP_SCALE = 0.125  # 1/sqrt(hd)
EXP_BIAS = -24.0  # exp(s/8 - 24): cancels in softmax, keeps fp32 range safe
GMAX = 2  # score-psum kt-tiles per exp instruction

F32 = mybir.dt.float32
BF16 = mybir.dt.bfloat16
BF16_NP = ml_dtypes.bfloat16


def _kt_groups(kt):
    groups = []
    k0 = 0
    while k0 < kt:
        g = min(GMAX, kt - k0)
        if (kt - k0) % GMAX == 1 and GMAX > 1:
            g = min(GMAX - 1, kt - k0)
        groups.append((k0, g))
        k0 += g
    return groups


def _perm_matrix():
    """lhsT for the rotate_half matmul: qrot^T = lhsT.T @ q^T."""
    mt = np.zeros((DL, DL), dtype=np.float32)
    for o in (0, HD):
        for r in range(HD // 2):
            mt[o + r, o + r + HD // 2] = -1.0
            mt[o + r + HD // 2, o + r] = 1.0
    return np.ascontiguousarray(mt.T)


def split_excess_waits(nc, max_waits=1):
    """This container's walrus rejects >1 semaphore wait per instruction;
    split excess waits onto NoOp carriers on the same engine."""
    for bb in nc.m.functions[0].blocks:
        insts = bb.instructions
        idx = 0
        while idx < len(insts):
            ins = insts[idx]
            si = ins.sync_info
            if si is not None and si.on_wait and len(si.on_wait) > max_waits:
                ow = list(si.on_wait)
                si.on_wait = ow[-max_waits:]
                extra = ow[:-max_waits]
                k = 0
                while extra:
                    chunk, extra = extra[:max_waits], extra[max_waits:]
                    c = mybir.InstNoOp(name=f"{ins.name}-ws{k}", ins=[], outs=[])
                    c.engine = ins.engine
                    c.sync_info = mybir.SyncInfo(on_wait=chunk, on_update=[])
                    nc.register_instruction(c)
                    insts.insert(idx, c)
                    idx += 1
                    k += 1
            idx += 1


def build_nc(b=2, s=2048, chunk=512, pt_bufs=10, debug=False):
    kt = s // 128
    nch = s // chunk
    dt8 = D // 128
    shard_half = s // N_CORES  # 256 tokens per core per batch
    groups = _kt_groups(kt)

    nc = bass.Bass()
    # all layout prep is host-side; everything below is bf16 device-ready
    xtp = nc.declare_dram_parameter("xt", [128, b * dt8, s], BF16, isOutput=False)
    csp = nc.declare_dram_parameter("csn", [128, s], BF16, isOutput=False)
    snp = nc.declare_dram_parameter("snn", [128, s], BF16, isOutput=False)
    wqp = nc.declare_dram_parameter("wq", [128, dt8, DL], BF16, isOutput=False)
    wkp = nc.declare_dram_parameter("wk", [128, dt8, DL], BF16, isOutput=False)
    wvp = nc.declare_dram_parameter("wv", [128, dt8, DL], BF16, isOutput=False)
    wop = nc.declare_dram_parameter("wo", [128, dt8, D], BF16, isOutput=False)
    selp = nc.declare_dram_parameter("sel", [H, N_CORES, 128], BF16, isOutput=False)
    mpp = nc.declare_dram_parameter("mperm", [DL, DL], BF16, isOutput=False)
    idp = nc.declare_dram_parameter("ident", [128, 128], BF16, isOutput=False)
    out = nc.declare_dram_parameter("out", [b * shard_half, D], F32, isOutput=True)
    if debug:
        dbg_q = nc.declare_dram_parameter("dbg_q", [b, DL, s], F32, isOutput=True)
        dbg_k = nc.declare_dram_parameter("dbg_k", [b, DL, s], F32, isOutput=True)
        dbg_v = nc.declare_dram_parameter("dbg_v", [b, DL, s], F32, isOutput=True)
        dbg_att = nc.declare_dram_parameter("dbg_att", [b, DL, s], F32, isOutput=True)

    with tile.TileContext(nc) as tc:
        with (
            tc.tile_pool(name="dram", bufs=1, space="DRAM") as dram,
            tc.tile_pool(name="const", bufs=1) as cpool,
            tc.tile_pool(name="xt", bufs=2) as xtpool,
            tc.tile_pool(name="qkv", bufs=2) as qkvpool,
            tc.tile_pool(name="rope", bufs=2) as ropepool,
            tc.tile_pool(name="pt", bufs=pt_bufs) as ptpool,
            tc.tile_pool(name="att", bufs=2) as attpool,
            tc.tile_pool(name="nrm", bufs=1) as nrmpool,
            tc.tile_pool(name="recv", bufs=1) as rcvpool,
            tc.tile_pool(name="outp", bufs=1) as outpool,
            # PSUM: 8 banks. psA = scores (2 tags x 2 banks; projections and
            # Wo borrow). psB = 2 PV banks. psC = 2 banks for v-transposes /
            # rot / bc broadcasts.
            tc.tile_pool(name="psA", bufs=1, space="PSUM") as psA,
            tc.tile_pool(name="psB", bufs=2, space="PSUM") as psB,
            tc.tile_pool(name="psC", bufs=2, space="PSUM") as psC,
        ):
            # ---------- constants (direct bf16 loads, no staging) ----------
            id_sb = cpool.tile([128, 128], BF16, tag="ident")
            nc.sync.dma_start(id_sb[:], idp[:])
            mp_sb = cpool.tile([DL, DL], BF16, tag="mperm")
            nc.sync.dma_start(mp_sb[:], mpp[:])

            # x^T for both batches (one big DMA each; batch 1's overlaps
            # batch-0 compute)
            xt0 = xtpool.tile([128, dt8, s], BF16, tag="xt", name="xt0")
            nc.sync.dma_start(xt0[:], xtp[:, 0:dt8, :])

            wq_sb = cpool.tile([128, dt8, DL], BF16, tag="wq")
            nc.sync.dma_start(wq_sb[:], wqp[:])
            wk_sb = cpool.tile([128, dt8, DL], BF16, tag="wk")
            nc.sync.dma_start(wk_sb[:], wkp[:])
            wv_sb = cpool.tile([128, dt8, DL], BF16, tag="wv")
            nc.sync.dma_start(wv_sb[:], wvp[:])
            cs128 = cpool.tile([128, s], BF16, tag="cs")
            nc.sync.dma_start(cs128[:], csp[:])
            sn128 = cpool.tile([128, s], BF16, tag="sn")
            nc.sync.dma_start(sn128[:], snp[:])
            sel_sb = cpool.tile([H, N_CORES, 128], BF16, tag="sel")
            nc.sync.dma_start(sel_sb[:], selp[:])

            xt1 = xtpool.tile([128, dt8, s], BF16, tag="xt", name="xt1")
            nc.gpsimd.dma_start(xt1[:], xtp[:, dt8 : 2 * dt8, :])

            biasc = cpool.tile([128, 1], F32, tag="biasc")
            nc.vector.memset(biasc[:], EXP_BIAS)

            wo_sb = cpool.tile([128, dt8, D], BF16, tag="wo")

            # ---------- pipeline pieces ----------
            def emit_proj(wsb, dst, ch, xt_sb, rope):
                cols = slice(ch * chunk, (ch + 1) * chunk)
                ps = psA.tile([128, chunk], F32, tag=f"sc{ch % 2}")
                for dt in range(dt8):
                    nc.tensor.matmul(
                        ps[:],
                        wsb[:, dt, :],
                        xt_sb[:, dt, cols],
                        start=(dt == 0),
                        stop=(dt == dt8 - 1),
                    )
                if not rope:
                    nc.vector.tensor_copy(dst[:, cols], ps[:])
                    return
                tsb = ropepool.tile([128, chunk], BF16, tag="tsb")
                nc.scalar.copy(tsb[:], ps[:])
                rps = psC.tile([128, chunk], F32, tag="tp")
                nc.tensor.matmul(rps[:], mp_sb[:], tsb[:], start=True, stop=True)
                m1 = ropepool.tile([128, chunk], BF16, tag="m1")
                nc.vector.tensor_tensor(
                    m1[:], tsb[:], cs128[:, cols], mybir.AluOpType.mult
                )
                m2 = ropepool.tile([128, chunk], BF16, tag="m2")
                nc.vector.tensor_tensor(
                    m2[:], rps[:], sn128[:, cols], mybir.AluOpType.mult
                )
                nc.vector.tensor_tensor(
                    dst[:, cols], m1[:], m2[:], mybir.AluOpType.add
                )

            def emit_vt_group(ch, vt_sb, v_sb):
                vps = psC.tile([128, 4, 128], BF16, tag="tp")
                for j in range(4):
                    ktt = ch * 4 + j
                    nc.tensor.transpose(
                        vps[:, j, :],
                        vt_sb[:, ktt * 128 : (ktt + 1) * 128],
                        id_sb[:],
                    )
                nc.vector.tensor_copy(
                    v_sb[:, ch * 4 : (ch + 1) * 4, :, 0:HD],
                    vps[:].rearrange("p t (h d) -> p t h d", h=HL),
                )

            def emit_attn_chunk(bi, ch, q_rope, k_rope, v_sb, aohs):
                cols = slice(ch * chunk, (ch + 1) * chunk)
                pts = {}
                for gi, (k0, glen) in enumerate(groups):
                    for h in range(HL):
                        rows = slice(h * HD, (h + 1) * HD)
                        sg = psA.tile([128, GMAX, chunk], F32, tag=f"sc{h}")
                        for j in range(glen):
                            ktt = k0 + j
                            nc.tensor.matmul(
                                sg[:, j, :],
                                k_rope[rows, ktt * 128 : (ktt + 1) * 128],
                                q_rope[rows, cols],
                                start=True,
                                stop=True,
                            )
                        pt = ptpool.tile([128, GMAX, chunk], BF16, tag="pt")
                        nc.scalar.activation(
                            pt[:, :glen, :],
                            sg[:, :glen, :],
                            mybir.ActivationFunctionType.Exp,
                            bias=biasc[:],
                            scale=EXP_SCALE,
                        )
                        pts[(gi, h)] = pt
                for h in range(HL):
                    pv = psB.tile([HD + 1, chunk], F32, tag="pv")
                    for gi, (k0, glen) in enumerate(groups):
                        pt = pts[(gi, h)]
                        for j in range(glen):
                            ktt = k0 + j
                            nc.tensor.matmul(
                                pv[:],
                                v_sb[:, ktt, h, :],
                                pt[:, j, :],
                                start=(ktt == 0),
                                stop=(ktt == kt - 1),
                            )
                    # unnormalized numerator + sigma row; 1/sigma applied
                    # once, consumer-side after the A2A
                    nc.vector.tensor_copy(aohs[h][:, cols], pv[:])

            # ---------- batch-0 QKV ----------
            q0 = qkvpool.tile([DL, s], BF16, tag="q_rope", bufs=1)
            k0_ = qkvpool.tile([DL, s], BF16, tag="k_rope")
            vt0 = qkvpool.tile([DL, s], BF16, tag="vt", bufs=1)
            v0 = qkvpool.tile([128, kt, HL, HD + 1], BF16, tag="v_sb")
            nc.vector.memset(v0[:, :, :, HD : HD + 1], 1.0)
            for ch in range(nch):
                emit_proj(wk_sb, k0_, ch, xt0, rope=True)
                emit_proj(wv_sb, vt0, ch, xt0, rope=False)
                emit_vt_group(ch, vt0, v0)
            for ch in range(nch):
                emit_proj(wq_sb, q0, ch, xt0, rope=True)

            # Wo load: off the critical path, overlaps batch-0 attention
            nc.sync.dma_start(wo_sb[:], wop[:])

            # ---------- batch-0 attention, batch-1 kv interleaved ----------
            ao0 = [
                attpool.tile([HD + 1, s], BF16, tag=f"aoh{h}", name=f"ao0_{h}")
                for h in range(HL)
            ]
            q1 = qkvpool.tile([DL, s], BF16, tag="q_rope", bufs=1)
            k1 = qkvpool.tile([DL, s], BF16, tag="k_rope")
            vt1 = qkvpool.tile([DL, s], BF16, tag="vt", bufs=1)
            v1 = qkvpool.tile([128, kt, HL, HD + 1], BF16, tag="v_sb")
            for ch in range(nch):
                emit_attn_chunk(0, ch, q0, k0_, v0, ao0)
                if ch == 0:
                    nc.vector.memset(v1[:, :, :, HD : HD + 1], 1.0)
                emit_proj(wk_sb, k1, ch, xt1, rope=True)
                emit_proj(wv_sb, vt1, ch, xt1, rope=False)
                emit_vt_group(ch, vt1, v1)

            # ---------- A2A / Wo ----------
            def emit_a2a(aohs, col0, w, tag):
                """AllToAll of tokens [col0, col0 + 8*w) (w per peer).
                rows 0..127: attn dims (h0, h1); rows 128..129: sigma."""
                a2a_in = dram.tile(
                    [N_CORES, DL + HL, w], BF16, tag=f"a2a_in{tag}",
                    name=f"a2a_in{tag}",
                )
                a2a_out = dram.tile(
                    [N_CORES, DL + HL, w], BF16, tag=f"a2a_out{tag}",
                    name=f"a2a_out{tag}",
                )
                for h in range(HL):
                    nc.sync.dma_start(
                        a2a_in[:, h * HD : (h + 1) * HD, :].rearrange(
                            "j r c -> r j c"
                        ),
                        aohs[h][0:HD, col0 : col0 + N_CORES * w].rearrange(
                            "r (j c) -> r j c", j=N_CORES
                        ),
                    )
                    nc.sync.dma_start(
                        a2a_in[:, DL + h : DL + h + 1, :].rearrange("j r c -> r j c"),
                        aohs[h][HD : HD + 1, col0 : col0 + N_CORES * w].rearrange(
                            "r (j c) -> r j c", j=N_CORES
                        ),
                    )
                nc.gpsimd.collective_compute(
                    "AllToAll",
                    mybir.AluOpType.bypass,
                    replica_groups=[list(range(N_CORES))],
                    ins=[a2a_in.opt()],
                    outs=[a2a_out.opt()],
                )
                return a2a_out

            def emit_wo(a2a_out, w, out_row0, tg):
                recv = rcvpool.tile(
                    [DL, N_CORES, w], BF16, tag=f"recv{tg}", name="recv"
                )
                nc.sync.dma_start(
                    recv[:], a2a_out[:, 0:DL, :].rearrange("j r c -> r j c")
                )
                # sigr row h*8+i = sigma of source core i's local head h
                sigr = rcvpool.tile([H, w], BF16, tag=f"sigr{tg}", name="sigr")
                for h in range(HL):
                    nc.sync.dma_start(
                        sigr[h * N_CORES : (h + 1) * N_CORES, :],
                        a2a_out[:, DL + h, :],
                    )
                sigf = nrmpool.tile([H, w], F32, tag=f"sigf{tg}", name="sigf")
                nc.vector.tensor_copy(sigf[:], sigr[:])
                rcpf = nrmpool.tile([H, w], F32, tag=f"rcpf{tg}", name="rcpf")
                nc.vector.reciprocal(rcpf[:], sigf[:])
                rcpb = nrmpool.tile([H, w], BF16, tag=f"rcpb{tg}", name="rcpb")
                nc.vector.tensor_copy(rcpb[:], rcpf[:])
                bcs = rcvpool.tile(
                    [DL, N_CORES, w], BF16, tag=f"bcs{tg}", name="bcs"
                )
                for i2 in range(N_CORES // 2):
                    bcp = psC.tile([128, 2, w], F32, tag="tp", name="bcp")
                    for k in range(2):
                        i = 2 * i2 + k
                        nc.tensor.matmul(
                            bcp[:, k, :],
                            sel_sb[:, i, :],
                            rcpb[:],
                            start=True,
                            stop=True,
                        )
                    nc.vector.tensor_copy(bcs[:, 2 * i2 : 2 * i2 + 2, :], bcp[:])
                nc.vector.tensor_tensor(
                    recv[:], recv[:], bcs[:], mybir.AluOpType.mult
                )
                for j in range(w // 128):
                    osb = outpool.tile([128, D], F32, tag="osb", name="osb")
                    for nco in range(D // chunk):
                        wps = psA.tile(
                            [128, chunk], F32, tag=f"sc{(j + nco) % 2}", name="wps"
                        )
                        for i in range(N_CORES):
                            nc.tensor.matmul(
                                wps[:],
                                recv[:, i, j * 128 : (j + 1) * 128],
                                wo_sb[:, i, nco * chunk : (nco + 1) * chunk],
                                start=(i == 0),
                                stop=(i == N_CORES - 1),
                            )
                        nc.scalar.copy(osb[:, nco * chunk : (nco + 1) * chunk], wps[:])
                    nc.sync.dma_start(
                        out[out_row0 + j * 128 : out_row0 + (j + 1) * 128, :],
                        osb[:],
                    )

            if debug:
                for name, tl in (("dbg_q", q0), ("dbg_k", k0_), ("dbg_v", vt0)):
                    for cch in range(nch):
                        df = outpool.tile([DL, chunk], F32, tag="dbgf")
                        nc.vector.tensor_copy(
                            df[:], tl[:, cch * chunk : (cch + 1) * chunk]
                        )
                        nc.sync.dma_start(
                            {"dbg_q": dbg_q, "dbg_k": dbg_k, "dbg_v": dbg_v}[name][0][
                                :, cch * chunk : (cch + 1) * chunk
                            ],
                            df[:],
                        )

            a2a_out0 = emit_a2a(ao0, 0, shard_half, "b0")

            # ---------- batch-1 q + attention ----------
            ao1 = [
                attpool.tile([HD + 1, s], BF16, tag=f"aoh{h}", name=f"ao1_{h}")
                for h in range(HL)
            ]
            a2a_out1a = None
            for ch in range(nch):
                emit_proj(wq_sb, q1, ch, xt1, rope=True)
                emit_attn_chunk(1, ch, q1, k1, v1, ao1)
                if ch == 1:
                    # first half of batch 1 ships now; hides under ch2-3
                    a2a_out1a = emit_a2a(ao1, 0, shard_half // 2, "b1a")

            if debug:
                for bi, ao in ((0, ao0), (1, ao1)):
                    for h in range(HL):
                        for cch in range(nch):
                            df = outpool.tile([HD, chunk], F32, tag="dbgf2")
                            nc.vector.tensor_copy(
                                df[:], ao[h][0:HD, cch * chunk : (cch + 1) * chunk]
                            )
                            nc.sync.dma_start(
                                dbg_att[
                                    bi,
                                    h * HD : (h + 1) * HD,
                                    cch * chunk : (cch + 1) * chunk,
                                ],
                                df[:],
                            )

            # last collective first, then ALL Wo work fills its latency
            a2a_out1b = emit_a2a(ao1, s // 2, shard_half // 2, "b1b")
            emit_wo(a2a_out0, shard_half, 0, "b0")
            emit_wo(a2a_out1a, shard_half // 2, shard_half, "b1a")
            emit_wo(a2a_out1b, shard_half // 2, shard_half + shard_half // 2, "b1b")

    split_excess_waits(nc)
    return nc


def _host_prep(x, cos, sin, b, s):
    """Device-ready layouts shared across cores."""
    bt = b * s
    # x^T in the projection's contraction layout: [128, b*dt8, s]
    xt = np.ascontiguousarray(x.reshape(bt, D).T.astype(BF16_NP))  # [D, b*s]
    xt = (
        xt.reshape(D // 128, 128, b, s)
        .transpose(1, 2, 0, 3)
        .reshape(128, b * (D // 128), s)
    )
    xt = np.ascontiguousarray(xt)
    # doubled, transposed rope tables [128, s]: row p = table[t, p % 32]
    csn = np.ascontiguousarray(np.tile(cos.T, (4, 1)).astype(BF16_NP))
    snn = np.ascontiguousarray(np.tile(sin.T, (4, 1)).astype(BF16_NP))
    # selector for the consumer-side 1/sigma broadcast (sigr is h-major)
    selm = np.zeros((H, N_CORES, 128), dtype=np.float32)
    for i in range(N_CORES):
        for p in range(128):
            selm[(p // HD) * N_CORES + i, i, p] = 1.0
    selb = np.ascontiguousarray(selm.astype(BF16_NP))
    mperm = np.ascontiguousarray(_perm_matrix().astype(BF16_NP))
    ident = np.ascontiguousarray(np.eye(128, dtype=np.float32).astype(BF16_NP))
    return xt, csn, snn, selb, mperm, ident


def _swz(w):  # [D, M] -> [128, dt8, M] bf16
    m = w.shape[1]
    return np.ascontiguousarray(
        np.asarray(w, dtype=np.float32)
        .reshape(D // 128, 128, m)
        .transpose(1, 0, 2)
        .astype(BF16_NP)
    )


def make_in_maps(x, cos, sin, Wq, Wk, Wv, Wo, b, s):
    xt, csn, snn, selb, mperm, ident = _host_prep(
        np.asarray(x, dtype=np.float32),
        np.asarray(cos, dtype=np.float32),
        np.asarray(sin, dtype=np.float32),
        b, s,
    )
    wo_s = _swz(Wo)
    in_maps = []
    for c in range(N_CORES):
        cs = slice(c * DL, (c + 1) * DL)
        in_maps.append(
            {
                "xt": xt,
                "csn": csn,
                "snn": snn,
                "wq": _swz(Wq[:, cs]),
                "wk": _swz(Wk[:, cs]),
                "wv": _swz(Wv[:, cs]),
                "wo": wo_s,
                "sel": selb,
                "mperm": mperm,
                "ident": ident,
            }
        )
    return in_maps


_NC_CACHE = {}


def run(x, cos, sin, Wq, Wk, Wv, Wo, trace=False, chunk=512, pt_bufs=10,
        debug=False):
    b, s, _ = x.shape
    key = (b, s, chunk, pt_bufs, debug)
    if key not in _NC_CACHE:
        _NC_CACHE[key] = build_nc(
            b=b, s=s, chunk=chunk, pt_bufs=pt_bufs, debug=debug
        )
    nc = _NC_CACHE[key]
    in_maps = make_in_maps(x, cos, sin, Wq, Wk, Wv, Wo, b, s)
    res = run_bass_kernel_spmd(nc, in_maps, list(range(N_CORES)), trace=trace)
    sh = s // N_CORES  # 256
    hh = sh // 2  # 128
    b0 = np.concatenate(
        [res.results[c]["out"][0:sh] for c in range(N_CORES)], axis=0
    )
    b1a = np.concatenate(
        [res.results[c]["out"][sh : sh + hh] for c in range(N_CORES)], axis=0
    )
    b1b = np.concatenate(
        [res.results[c]["out"][sh + hh : 2 * sh] for c in range(N_CORES)], axis=0
    )
    full = np.stack([b0, np.concatenate([b1a, b1b], axis=0)], axis=0)
    return full.reshape(b, s, D), res


def kernel(x, cos, sin, Wq, Wk, Wv, Wo):
    out, _ = run(
        np.asarray(x), np.asarray(cos), np.asarray(sin),
        np.asarray(Wq), np.asarray(Wk), np.asarray(Wv), np.asarray(Wo),
    )
    return out.astype(np.float32)


# revision 38
# speedup vs baseline: 1.0700x; 1.0206x over previous
"""Multi-head attention with RoPE on 8 Trainium2 NeuronCores (v3).

Problem: x[2,2048,1024] -> MHA(16 heads, hd=64, NeoX RoPE, non-causal) -> out.

Sharding: tensor-parallel over heads. Each core owns 2 heads. All input
layout work (x^T, bf16 casts, doubled cos/sin tables, weight swizzles,
per-core weight column slices) happens host-side in make_in_maps; the
device kernel is pure compute:

  - q^T,k^T (RoPE'd via a permutation matmul) and v^T projections from the
    pre-transposed x^T, full sequence per core,
  - flash-style attention with *transposed* scores [s_k, s_q]; the softmax
    denominator comes from a fused ones-column in V (constant bias inside
    the exp keeps fp32 range safe),
  - AllToAll redistributes unnormalized numerator + sigma rows, split in
    three (batch 0 | batch 1 first half | batch 1 second half) so only the
    last small collective is exposed; Wo matmuls fill its latency,
  - consumer-side 1/sigma via one reciprocal + selector-matmul broadcast,
  - local Wo matmul produces [256 b0 | 128+128 b1] token rows per core.

All matmuls run in bf16 (fp32 PSUM accumulation); rel-err tolerance 2e-2.
"""

import sys

sys.path.insert(0, "/opt/trn_rl_repo")

import numpy as np  # noqa: E402
import ml_dtypes  # noqa: E402

import concourse.bass as bass  # noqa: E402
import concourse.mybir as mybir  # noqa: E402
import concourse.tile as tile  # noqa: E402
from concourse.bass_utils import run_bass_kernel_spmd  # noqa: E402

N_CORES = 8
D = 1024
H = 16
HD = 64
HL = H // N_CORES  # local heads per core
DL = HL * HD  # 128 local attn dims
EXP_SCALE = 0.125  # 1/sqrt(hd)
EXP_BIAS = -24.0  # exp(s/8 - 24): cancels in softmax, keeps fp32 range safe
GMAX = 2  # score-psum kt-tiles per exp instruction

F32 = mybir.dt.float32
BF16 = mybir.dt.bfloat16
BF16_NP = ml_dtypes.bfloat16


def _kt_groups(kt):
    groups = []
    k0 = 0
    while k0 < kt:
        g = min(GMAX, kt - k0)
        if (kt - k0) % GMAX == 1 and GMAX > 1:
            g = min(GMAX - 1, kt - k0)
        groups.append((k0, g))
        k0 += g
    return groups


def _perm_matrix():
    """lhsT for the rotate_half matmul: qrot^T = lhsT.T @ q^T."""
    mt = np.zeros((DL, DL), dtype=np.float32)
    for o in (0, HD):
        for r in range(HD // 2):
            mt[o + r, o + r + HD // 2] = -1.0
            mt[o + r + HD // 2, o + r] = 1.0
    return np.ascontiguousarray(mt.T)


def split_excess_waits(nc, max_waits=1):
    """This container's walrus rejects >1 semaphore wait per instruction;
    split excess waits onto NoOp carriers on the same engine."""
    for bb in nc.m.functions[0].blocks:
        insts = bb.instructions
        idx = 0
        while idx < len(insts):
            ins = insts[idx]
            si = ins.sync_info
            if si is not None and si.on_wait and len(si.on_wait) > max_waits:
                ow = list(si.on_wait)
                si.on_wait = ow[-max_waits:]
                extra = ow[:-max_waits]
                k = 0
                while extra:
                    chunk, extra = extra[:max_waits], extra[max_waits:]
                    c = mybir.InstNoOp(name=f"{ins.name}-ws{k}", ins=[], outs=[])
                    c.engine = ins.engine
                    c.sync_info = mybir.SyncInfo(on_wait=chunk, on_update=[])
                    nc.register_instruction(c)
                    insts.insert(idx, c)
                    idx += 1
                    k += 1
            idx += 1


def build_nc(b=2, s=2048, chunk=512, pt_bufs=10, debug=False):
    kt = s // 128
    nch = s // chunk
    dt8 = D // 128
    shard_half = s // N_CORES  # 256 tokens per core per batch
    groups = _kt_groups(kt)

    nc = bass.Bass()
    # all layout prep is host-side; everything below is bf16 device-ready
    xtp = nc.declare_dram_parameter("xt", [128, b * dt8, s], BF16, isOutput=False)
    csp = nc.declare_dram_parameter("csn", [128, s], BF16, isOutput=False)
    snp = nc.declare_dram_parameter("snn", [128, s], BF16, isOutput=False)
    wqp = nc.declare_dram_parameter("wq", [128, dt8, DL], BF16, isOutput=False)
    wkp = nc.declare_dram_parameter("wk", [128, dt8, DL], BF16, isOutput=False)
    wvp = nc.declare_dram_parameter("wv", [128, dt8, DL], BF16, isOutput=False)
    wop = nc.declare_dram_parameter("wo", [128, dt8, D], BF16, isOutput=False)
    selp = nc.declare_dram_parameter("sel", [H, N_CORES, 128], BF16, isOutput=False)
    mpp = nc.declare_dram_parameter("mperm", [DL, DL], BF16, isOutput=False)
    idp = nc.declare_dram_parameter("ident", [128, 128], BF16, isOutput=False)
    out = nc.declare_dram_parameter("out", [b * shard_half, D], F32, isOutput=True)
    if debug:
        dbg_q = nc.declare_dram_parameter("dbg_q", [b, DL, s], F32, isOutput=True)
        dbg_k = nc.declare_dram_parameter("dbg_k", [b, DL, s], F32, isOutput=True)
        dbg_v = nc.declare_dram_parameter("dbg_v", [b, DL, s], F32, isOutput=True)
        dbg_att = nc.declare_dram_parameter("dbg_att", [b, DL, s], F32, isOutput=True)

    with tile.TileContext(nc) as tc:
        with (
            tc.tile_pool(name="dram", bufs=1, space="DRAM") as dram,
            tc.tile_pool(name="const", bufs=1) as cpool,
            tc.tile_pool(name="xt", bufs=2) as xtpool,
            tc.tile_pool(name="qkv", bufs=2) as qkvpool,
            tc.tile_pool(name="rope", bufs=2) as ropepool,
            tc.tile_pool(name="pt", bufs=pt_bufs) as ptpool,
            tc.tile_pool(name="att", bufs=2) as attpool,
            tc.tile_pool(name="nrm", bufs=1) as nrmpool,
            tc.tile_pool(name="recv", bufs=1) as rcvpool,
            tc.tile_pool(name="outp", bufs=1) as outpool,
            # PSUM: 8 banks. psA = scores (2 tags x 2 banks; projections and
            # Wo borrow). psB = 2 PV banks. psC = 2 banks for v-transposes /
            # rot / bc broadcasts.
            tc.tile_pool(name="psA", bufs=1, space="PSUM") as psA,
            tc.tile_pool(name="psB", bufs=2, space="PSUM") as psB,
            tc.tile_pool(name="psC", bufs=2, space="PSUM") as psC,
        ):
            # ---------- constants (direct bf16 loads, no staging) ----------
            id_sb = cpool.tile([128, 128], BF16, tag="ident")
            nc.sync.dma_start(id_sb[:], idp[:])
            mp_sb = cpool.tile([DL, DL], BF16, tag="mperm")
            nc.sync.dma_start(mp_sb[:], mpp[:])

            # x^T for both batches (one big DMA each; batch 1's overlaps
            # batch-0 compute)
            xt0 = xtpool.tile([128, dt8, s], BF16, tag="xt", name="xt0")
            nc.sync.dma_start(xt0[:], xtp[:, 0:dt8, :])

            wq_sb = cpool.tile([128, dt8, DL], BF16, tag="wq")
            nc.sync.dma_start(wq_sb[:], wqp[:])
            wk_sb = cpool.tile([128, dt8, DL], BF16, tag="wk")
            nc.sync.dma_start(wk_sb[:], wkp[:])
            wv_sb = cpool.tile([128, dt8, DL], BF16, tag="wv")
            nc.sync.dma_start(wv_sb[:], wvp[:])
            cs128 = cpool.tile([128, s], BF16, tag="cs")
            nc.sync.dma_start(cs128[:], csp[:])
            sn128 = cpool.tile([128, s], BF16, tag="sn")
            nc.sync.dma_start(sn128[:], snp[:])
            sel_sb = cpool.tile([H, N_CORES, 128], BF16, tag="sel")
            nc.sync.dma_start(sel_sb[:], selp[:])

            xt1 = xtpool.tile([128, dt8, s], BF16, tag="xt", name="xt1")
            nc.gpsimd.dma_start(xt1[:], xtp[:, dt8 : 2 * dt8, :])

            biasc = cpool.tile([128, 1], F32, tag="biasc")
            nc.vector.memset(biasc[:], EXP_BIAS)

            wo_sb = cpool.tile([128, dt8, D], BF16, tag="wo")

            # ---------- pipeline pieces ----------
            def emit_proj(wsb, dst, ch, xt_sb, rope):
                cols = slice(ch * chunk, (ch + 1) * chunk)
                ps = psC.tile([128, chunk], F32, tag="tp", name="proj_ps")
                for dt in range(dt8):
                    nc.tensor.matmul(
                        ps[:],
                        wsb[:, dt, :],
                        xt_sb[:, dt, cols],
                        start=(dt == 0),
                        stop=(dt == dt8 - 1),
                    )
                if not rope:
                    nc.vector.tensor_copy(dst[:, cols], ps[:])
                    return
                tsb = ropepool.tile([128, chunk], BF16, tag="tsb")
                nc.scalar.copy(tsb[:], ps[:])
                rps = psC.tile([128, chunk], F32, tag="tp")
                nc.tensor.matmul(rps[:], mp_sb[:], tsb[:], start=True, stop=True)
                m1 = ropepool.tile([128, chunk], BF16, tag="m1")
                nc.vector.tensor_tensor(
                    m1[:], tsb[:], cs128[:, cols], mybir.AluOpType.mult
                )
                m2 = ropepool.tile([128, chunk], BF16, tag="m2")
                nc.vector.tensor_tensor(
                    m2[:], rps[:], sn128[:, cols], mybir.AluOpType.mult
                )
                nc.vector.tensor_tensor(
                    dst[:, cols], m1[:], m2[:], mybir.AluOpType.add
                )

            def emit_vt_group(ch, vt_sb, v_sb):
                vps = psC.tile([128, 4, 128], BF16, tag="tp")
                for j in range(4):
                    ktt = ch * 4 + j
                    nc.tensor.transpose(
                        vps[:, j, :],
                        vt_sb[:, ktt * 128 : (ktt + 1) * 128],
                        id_sb[:],
                    )
                nc.vector.tensor_copy(
                    v_sb[:, ch * 4 : (ch + 1) * 4, :, 0:HD],
                    vps[:].rearrange("p t (h d) -> p t h d", h=HL),
                )

            def emit_attn_chunk(bi, ch, q_rope, k_rope, v_sb, aohs):
                cols = slice(ch * chunk, (ch + 1) * chunk)
                pts = {}
                for gi, (k0, glen) in enumerate(groups):
                    for h in range(HL):
                        rows = slice(h * HD, (h + 1) * HD)
                        sg = psA.tile([128, GMAX, chunk], F32, tag=f"sc{h}")
                        for j in range(glen):
                            ktt = k0 + j
                            nc.tensor.matmul(
                                sg[:, j, :],
                                k_rope[rows, ktt * 128 : (ktt + 1) * 128],
                                q_rope[rows, cols],
                                start=True,
                                stop=True,
                            )
                        pt = ptpool.tile([128, GMAX, chunk], BF16, tag="pt")
                        nc.scalar.activation(
                            pt[:, :glen, :],
                            sg[:, :glen, :],
                            mybir.ActivationFunctionType.Exp,
                            bias=biasc[:],
                            scale=EXP_SCALE,
                        )
                        pts[(gi, h)] = pt
                for h in range(HL):
                    pv = psB.tile([HD + 1, chunk], F32, tag="pv")
                    for gi, (k0, glen) in enumerate(groups):
                        pt = pts[(gi, h)]
                        for j in range(glen):
                            ktt = k0 + j
                            nc.tensor.matmul(
                                pv[:],
                                v_sb[:, ktt, h, :],
                                pt[:, j, :],
                                start=(ktt == 0),
                                stop=(ktt == kt - 1),
                            )
                    # unnormalized numerator + sigma row; 1/sigma applied
                    # once, consumer-side after the A2A
                    nc.vector.tensor_copy(aohs[h][:, cols], pv[:])

            # ---------- batch-0 QKV ----------
            q0 = qkvpool.tile([DL, s], BF16, tag="q_rope", bufs=1)
            k0_ = qkvpool.tile([DL, s], BF16, tag="k_rope")
            vt0 = qkvpool.tile([DL, s], BF16, tag="vt", bufs=1)
            v0 = qkvpool.tile([128, kt, HL, HD + 1], BF16, tag="v_sb")
            nc.vector.memset(v0[:, :, :, HD : HD + 1], 1.0)
            for ch in range(nch):
                emit_proj(wk_sb, k0_, ch, xt0, rope=True)
                emit_proj(wv_sb, vt0, ch, xt0, rope=False)
                emit_vt_group(ch, vt0, v0)
            for ch in range(nch):
                emit_proj(wq_sb, q0, ch, xt0, rope=True)

            # Wo load: off the critical path, overlaps batch-0 attention
            nc.sync.dma_start(wo_sb[:], wop[:])

            # ---------- batch-0 attention, batch-1 kv interleaved ----------
            ao0 = [
                attpool.tile([HD + 1, s], BF16, tag=f"aoh{h}", name=f"ao0_{h}")
                for h in range(HL)
            ]
            q1 = qkvpool.tile([DL, s], BF16, tag="q_rope", bufs=1)
            k1 = qkvpool.tile([DL, s], BF16, tag="k_rope")
            vt1 = qkvpool.tile([DL, s], BF16, tag="vt", bufs=1)
            v1 = qkvpool.tile([128, kt, HL, HD + 1], BF16, tag="v_sb")
            for ch in range(nch):
                emit_attn_chunk(0, ch, q0, k0_, v0, ao0)
                if ch == 0:
                    nc.vector.memset(v1[:, :, :, HD : HD + 1], 1.0)
                emit_proj(wk_sb, k1, ch, xt1, rope=True)
                emit_proj(wv_sb, vt1, ch, xt1, rope=False)
                emit_vt_group(ch, vt1, v1)

            # ---------- A2A / Wo ----------
            def emit_a2a(aohs, col0, w, tag):
                """AllToAll of tokens [col0, col0 + 8*w) (w per peer).
                rows 0..127: attn dims (h0, h1); rows 128..129: sigma."""
                a2a_in = dram.tile(
                    [N_CORES, DL + HL, w], BF16, tag=f"a2a_in{tag}",
                    name=f"a2a_in{tag}",
                )
                a2a_out = dram.tile(
                    [N_CORES, DL + HL, w], BF16, tag=f"a2a_out{tag}",
                    name=f"a2a_out{tag}",
                )
                for h in range(HL):
                    nc.sync.dma_start(
                        a2a_in[:, h * HD : (h + 1) * HD, :].rearrange(
                            "j r c -> r j c"
                        ),
                        aohs[h][0:HD, col0 : col0 + N_CORES * w].rearrange(
                            "r (j c) -> r j c", j=N_CORES
                        ),
                    )
                    nc.sync.dma_start(
                        a2a_in[:, DL + h : DL + h + 1, :].rearrange("j r c -> r j c"),
                        aohs[h][HD : HD + 1, col0 : col0 + N_CORES * w].rearrange(
                            "r (j c) -> r j c", j=N_CORES
                        ),
                    )
                nc.gpsimd.collective_compute(
                    "AllToAll",
                    mybir.AluOpType.bypass,
                    replica_groups=[list(range(N_CORES))],
                    ins=[a2a_in.opt()],
                    outs=[a2a_out.opt()],
                )
                return a2a_out

            def emit_wo(a2a_out, w, out_row0, tg):
                recv = rcvpool.tile(
                    [DL, N_CORES, w], BF16, tag=f"recv{tg}", name="recv"
                )
                nc.sync.dma_start(
                    recv[:], a2a_out[:, 0:DL, :].rearrange("j r c -> r j c")
                )
                # sigr row h*8+i = sigma of source core i's local head h
                sigr = rcvpool.tile([H, w], BF16, tag=f"sigr{tg}", name="sigr")
                for h in range(HL):
                    nc.sync.dma_start(
                        sigr[h * N_CORES : (h + 1) * N_CORES, :],
                        a2a_out[:, DL + h, :],
                    )
                sigf = nrmpool.tile([H, w], F32, tag=f"sigf{tg}", name="sigf")
                nc.vector.tensor_copy(sigf[:], sigr[:])
                rcpf = nrmpool.tile([H, w], F32, tag=f"rcpf{tg}", name="rcpf")
                nc.vector.reciprocal(rcpf[:], sigf[:])
                rcpb = nrmpool.tile([H, w], BF16, tag=f"rcpb{tg}", name="rcpb")
                nc.vector.tensor_copy(rcpb[:], rcpf[:])
                bcs = rcvpool.tile(
                    [DL, N_CORES, w], BF16, tag=f"bcs{tg}", name="bcs"
                )
                for i2 in range(N_CORES // 2):
                    bcp = psC.tile([128, 2, w], F32, tag="tp", name="bcp")
                    for k in range(2):
                        i = 2 * i2 + k
                        nc.tensor.matmul(
                            bcp[:, k, :],
                            sel_sb[:, i, :],
                            rcpb[:],
                            start=True,
                            stop=True,
                        )
                    nc.vector.tensor_copy(bcs[:, 2 * i2 : 2 * i2 + 2, :], bcp[:])
                nc.vector.tensor_tensor(
                    recv[:], recv[:], bcs[:], mybir.AluOpType.mult
                )
                for j in range(w // 128):
                    osb = outpool.tile([128, D], F32, tag="osb", name="osb")
                    for nco in range(D // chunk):
                        wps = psA.tile(
                            [128, chunk], F32, tag=f"sc{(j + nco) % 2}", name="wps"
                        )
                        for i in range(N_CORES):
                            nc.tensor.matmul(
                                wps[:],
                                recv[:, i, j * 128 : (j + 1) * 128],
                                wo_sb[:, i, nco * chunk : (nco + 1) * chunk],
                                start=(i == 0),
                                stop=(i == N_CORES - 1),
                            )
                        nc.scalar.copy(osb[:, nco * chunk : (nco + 1) * chunk], wps[:])
                    nc.sync.dma_start(
                        out[out_row0 + j * 128 : out_row0 + (j + 1) * 128, :],
                        osb[:],
                    )

            if debug:
                for name, tl in (("dbg_q", q0), ("dbg_k", k0_), ("dbg_v", vt0)):
                    for cch in range(nch):
                        df = outpool.tile([DL, chunk], F32, tag="dbgf")
                        nc.vector.tensor_copy(
                            df[:], tl[:, cch * chunk : (cch + 1) * chunk]
                        )
                        nc.sync.dma_start(
                            {"dbg_q": dbg_q, "dbg_k": dbg_k, "dbg_v": dbg_v}[name][0][
                                :, cch * chunk : (cch + 1) * chunk
                            ],
                            df[:],
                        )

            a2a_out0 = emit_a2a(ao0, 0, shard_half, "b0")

            # ---------- batch-1 q + attention ----------
            ao1 = [
                attpool.tile([HD + 1, s], BF16, tag=f"aoh{h}", name=f"ao1_{h}")
                for h in range(HL)
            ]
            a2a_out1a = None
            emit_proj(wq_sb, q1, 0, xt1, rope=True)
            emit_proj(wq_sb, q1, 1, xt1, rope=True)
            for ch in range(nch):
                if ch + 2 < nch:
                    emit_proj(wq_sb, q1, ch + 2, xt1, rope=True)
                emit_attn_chunk(1, ch, q1, k1, v1, ao1)
                if ch == 1:
                    # first half of batch 1 ships now; hides under ch2-3
                    a2a_out1a = emit_a2a(ao1, 0, shard_half // 2, "b1a")

            if debug:
                for bi, ao in ((0, ao0), (1, ao1)):
                    for h in range(HL):
                        for cch in range(nch):
                            df = outpool.tile([HD, chunk], F32, tag="dbgf2")
                            nc.vector.tensor_copy(
                                df[:], ao[h][0:HD, cch * chunk : (cch + 1) * chunk]
                            )
                            nc.sync.dma_start(
                                dbg_att[
                                    bi,
                                    h * HD : (h + 1) * HD,
                                    cch * chunk : (cch + 1) * chunk,
                                ],
                                df[:],
                            )

            # last collective first, then ALL Wo work fills its latency
            a2a_out1b = emit_a2a(ao1, s // 2, shard_half // 2, "b1b")
            emit_wo(a2a_out0, shard_half, 0, "b0")
            emit_wo(a2a_out1a, shard_half // 2, shard_half, "b1a")
            emit_wo(a2a_out1b, shard_half // 2, shard_half + shard_half // 2, "b1b")

    split_excess_waits(nc)
    return nc


def _host_prep(x, cos, sin, b, s):
    """Device-ready layouts shared across cores."""
    bt = b * s
    # x^T in the projection's contraction layout: [128, b*dt8, s]
    xt = np.ascontiguousarray(x.reshape(bt, D).T.astype(BF16_NP))  # [D, b*s]
    xt = (
        xt.reshape(D // 128, 128, b, s)
        .transpose(1, 2, 0, 3)
        .reshape(128, b * (D // 128), s)
    )
    xt = np.ascontiguousarray(xt)
    # doubled, transposed rope tables [128, s]: row p = table[t, p % 32]
    csn = np.ascontiguousarray(np.tile(cos.T, (4, 1)).astype(BF16_NP))
    snn = np.ascontiguousarray(np.tile(sin.T, (4, 1)).astype(BF16_NP))
    # selector for the consumer-side 1/sigma broadcast (sigr is h-major)
    selm = np.zeros((H, N_CORES, 128), dtype=np.float32)
    for i in range(N_CORES):
        for p in range(128):
            selm[(p // HD) * N_CORES + i, i, p] = 1.0
    selb = np.ascontiguousarray(selm.astype(BF16_NP))
    mperm = np.ascontiguousarray(_perm_matrix().astype(BF16_NP))
    ident = np.ascontiguousarray(np.eye(128, dtype=np.float32).astype(BF16_NP))
    return xt, csn, snn, selb, mperm, ident


def _swz(w):  # [D, M] -> [128, dt8, M] bf16
    m = w.shape[1]
    return np.ascontiguousarray(
        np.asarray(w, dtype=np.float32)
        .reshape(D // 128, 128, m)
        .transpose(1, 0, 2)
        .astype(BF16_NP)
    )


def make_in_maps(x, cos, sin, Wq, Wk, Wv, Wo, b, s):
    xt, csn, snn, selb, mperm, ident = _host_prep(
        np.asarray(x, dtype=np.float32),
        np.asarray(cos, dtype=np.float32),
        np.asarray(sin, dtype=np.float32),
        b, s,
    )
    wo_s = _swz(Wo)
    in_maps = []
    for c in range(N_CORES):
        cs = slice(c * DL, (c + 1) * DL)
        in_maps.append(
            {
                "xt": xt,
                "csn": csn,
                "snn": snn,
                "wq": _swz(Wq[:, cs]),
                "wk": _swz(Wk[:, cs]),
                "wv": _swz(Wv[:, cs]),
                "wo": wo_s,
                "sel": selb,
                "mperm": mperm,
                "ident": ident,
            }
        )
    return in_maps


_NC_CACHE = {}


def run(x, cos, sin, Wq, Wk, Wv, Wo, trace=False, chunk=512, pt_bufs=10,
        debug=False):
    b, s, _ = x.shape
    key = (b, s, chunk, pt_bufs, debug)
    if key not in _NC_CACHE:
        _NC_CACHE[key] = build_nc(
            b=b, s=s, chunk=chunk, pt_bufs=pt_bufs, debug=debug
        )
    nc = _NC_CACHE[key]
    in_maps = make_in_maps(x, cos, sin, Wq, Wk, Wv, Wo, b, s)
    res = run_bass_kernel_spmd(nc, in_maps, list(range(N_CORES)), trace=trace)
    sh = s // N_CORES  # 256
    hh = sh // 2  # 128
    b0 = np.concatenate(
        [res.results[c]["out"][0:sh] for c in range(N_CORES)], axis=0
    )
    b1a = np.concatenate(
        [res.results[c]["out"][sh : sh + hh] for c in range(N_CORES)], axis=0
    )
    b1b = np.concatenate(
        [res.results[c]["out"][sh + hh : 2 * sh] for c in range(N_CORES)], axis=0
    )
    full = np.stack([b0, np.concatenate([b1a, b1b], axis=0)], axis=0)
    return full.reshape(b, s, D), res


def kernel(x, cos, sin, Wq, Wk, Wv, Wo):
    out, _ = run(
        np.asarray(x), np.asarray(cos), np.asarray(sin),
        np.asarray(Wq), np.asarray(Wk), np.asarray(Wv), np.asarray(Wo),
    )
    return out.astype(np.float32)


# revision 39
# speedup vs baseline: 1.3426x; 1.2548x over previous
"""Multi-head attention with RoPE on 8 Trainium2 NeuronCores (v3).

Problem: x[2,2048,1024] -> MHA(16 heads, hd=64, NeoX RoPE, non-causal) -> out.

Sharding: tensor-parallel over heads. Each core owns 2 heads. All input
layout work (x^T, bf16 casts, doubled cos/sin tables, weight swizzles,
per-core weight column slices) happens host-side in make_in_maps; the
device kernel is pure compute:

  - q^T,k^T (RoPE'd via a permutation matmul) and v^T projections from the
    pre-transposed x^T, full sequence per core,
  - flash-style attention with *transposed* scores [s_k, s_q]; the softmax
    denominator comes from a fused ones-column in V (constant bias inside
    the exp keeps fp32 range safe),
  - AllToAll redistributes unnormalized numerator + sigma rows, split in
    three (batch 0 | batch 1 first half | batch 1 second half) so only the
    last small collective is exposed; Wo matmuls fill its latency,
  - consumer-side 1/sigma via one reciprocal + selector-matmul broadcast,
  - local Wo matmul produces [256 b0 | 128+128 b1] token rows per core.

All matmuls run in bf16 (fp32 PSUM accumulation); rel-err tolerance 2e-2.
"""

import sys

sys.path.insert(0, "/opt/trn_rl_repo")

import numpy as np  # noqa: E402
import ml_dtypes  # noqa: E402

import concourse.bass as bass  # noqa: E402
import concourse.mybir as mybir  # noqa: E402
import concourse.tile as tile  # noqa: E402
from concourse.bass_utils import run_bass_kernel_spmd  # noqa: E402

N_CORES = 8
D = 1024
H = 16
HD = 64
HL = H // N_CORES  # local heads per core
DL = HL * HD  # 128 local attn dims
EXP_SCALE = 0.125  # 1/sqrt(hd)
EXP_BIAS = -24.0  # exp(s/8 - 24): cancels in softmax, keeps fp32 range safe
GMAX = 2  # score-psum kt-tiles per exp instruction

F32 = mybir.dt.float32
BF16 = mybir.dt.bfloat16
BF16_NP = ml_dtypes.bfloat16


def _kt_groups(kt):
    groups = []
    k0 = 0
    while k0 < kt:
        g = min(GMAX, kt - k0)
        if (kt - k0) % GMAX == 1 and GMAX > 1:
            g = min(GMAX - 1, kt - k0)
        groups.append((k0, g))
        k0 += g
    return groups


def _perm_matrix():
    """lhsT for the rotate_half matmul: qrot^T = lhsT.T @ q^T."""
    mt = np.zeros((DL, DL), dtype=np.float32)
    for o in (0, HD):
        for r in range(HD // 2):
            mt[o + r, o + r + HD // 2] = -1.0
            mt[o + r + HD // 2, o + r] = 1.0
    return np.ascontiguousarray(mt.T)


def split_excess_waits(nc, max_waits=1):
    """This container's walrus rejects >1 semaphore wait per instruction;
    split excess waits onto NoOp carriers on the same engine."""
    for bb in nc.m.functions[0].blocks:
        insts = bb.instructions
        idx = 0
        while idx < len(insts):
            ins = insts[idx]
            si = ins.sync_info
            if si is not None and si.on_wait and len(si.on_wait) > max_waits:
                ow = list(si.on_wait)
                si.on_wait = ow[-max_waits:]
                extra = ow[:-max_waits]
                k = 0
                while extra:
                    chunk, extra = extra[:max_waits], extra[max_waits:]
                    c = mybir.InstNoOp(name=f"{ins.name}-ws{k}", ins=[], outs=[])
                    c.engine = ins.engine
                    c.sync_info = mybir.SyncInfo(on_wait=chunk, on_update=[])
                    nc.register_instruction(c)
                    insts.insert(idx, c)
                    idx += 1
                    k += 1
            idx += 1


def build_nc(b=2, s=2048, chunk=512, pt_bufs=10, debug=False):
    kt = s // 128
    nch = s // chunk
    dt8 = D // 128
    shard_half = s // N_CORES  # 256 tokens per core per batch
    groups = _kt_groups(kt)

    nc = bass.Bass()
    # all layout prep is host-side; everything below is bf16 device-ready
    xtp = nc.declare_dram_parameter("xt", [128, b * dt8, s], BF16, isOutput=False)
    csp = nc.declare_dram_parameter("csn", [128, s], BF16, isOutput=False)
    snp = nc.declare_dram_parameter("snn", [128, s], BF16, isOutput=False)
    wqp = nc.declare_dram_parameter("wq", [128, dt8, DL], BF16, isOutput=False)
    wkp = nc.declare_dram_parameter("wk", [128, dt8, DL], BF16, isOutput=False)
    wvp = nc.declare_dram_parameter("wv", [128, dt8, DL], BF16, isOutput=False)
    wop = nc.declare_dram_parameter("wo", [128, dt8, D], BF16, isOutput=False)
    selp = nc.declare_dram_parameter("sel", [H, N_CORES, 128], BF16, isOutput=False)
    mpp = nc.declare_dram_parameter("mperm", [DL, DL], BF16, isOutput=False)
    idp = nc.declare_dram_parameter("ident", [128, 128], BF16, isOutput=False)
    out = nc.declare_dram_parameter("out", [b * shard_half, D], F32, isOutput=True)
    if debug:
        dbg_q = nc.declare_dram_parameter("dbg_q", [b, DL, s], F32, isOutput=True)
        dbg_k = nc.declare_dram_parameter("dbg_k", [b, DL, s], F32, isOutput=True)
        dbg_v = nc.declare_dram_parameter("dbg_v", [b, DL, s], F32, isOutput=True)
        dbg_att = nc.declare_dram_parameter("dbg_att", [b, DL, s], F32, isOutput=True)

    with tile.TileContext(nc) as tc:
        with (
            tc.tile_pool(name="dram", bufs=1, space="DRAM") as dram,
            tc.tile_pool(name="const", bufs=1) as cpool,
            tc.tile_pool(name="xt", bufs=2) as xtpool,
            tc.tile_pool(name="qkv", bufs=2) as qkvpool,
            tc.tile_pool(name="rope", bufs=2) as ropepool,
            tc.tile_pool(name="pt", bufs=pt_bufs) as ptpool,
            tc.tile_pool(name="att", bufs=2) as attpool,
            tc.tile_pool(name="nrm", bufs=1) as nrmpool,
            tc.tile_pool(name="recv", bufs=1) as rcvpool,
            tc.tile_pool(name="outp", bufs=1) as outpool,
            # PSUM: 8 banks. psA = scores (2 tags x 2 banks; projections and
            # Wo borrow). psB = 2 PV banks. psC = 2 banks for v-transposes /
            # rot / bc broadcasts.
            tc.tile_pool(name="psA", bufs=1, space="PSUM") as psA,
            tc.tile_pool(name="psB", bufs=2, space="PSUM") as psB,
            tc.tile_pool(name="psC", bufs=2, space="PSUM") as psC,
        ):
            # ---------- constants (direct bf16 loads, no staging) ----------
            id_sb = cpool.tile([128, 128], BF16, tag="ident")
            nc.sync.dma_start(id_sb[:], idp[:])
            mp_sb = cpool.tile([DL, DL], BF16, tag="mperm")
            nc.sync.dma_start(mp_sb[:], mpp[:])

            # x^T for both batches (one big DMA each; batch 1's overlaps
            # batch-0 compute)
            xt0 = xtpool.tile([128, dt8, s], BF16, tag="xt", name="xt0")
            nc.sync.dma_start(xt0[:], xtp[:, 0:dt8, :])

            wq_sb = cpool.tile([128, dt8, DL], BF16, tag="wq")
            nc.sync.dma_start(wq_sb[:], wqp[:])
            wk_sb = cpool.tile([128, dt8, DL], BF16, tag="wk")
            nc.sync.dma_start(wk_sb[:], wkp[:])
            wv_sb = cpool.tile([128, dt8, DL], BF16, tag="wv")
            nc.sync.dma_start(wv_sb[:], wvp[:])
            cs128 = cpool.tile([128, s], BF16, tag="cs")
            nc.sync.dma_start(cs128[:], csp[:])
            sn128 = cpool.tile([128, s], BF16, tag="sn")
            nc.sync.dma_start(sn128[:], snp[:])
            sel_sb = cpool.tile([H, N_CORES, 128], BF16, tag="sel")
            nc.sync.dma_start(sel_sb[:], selp[:])

            xt1 = xtpool.tile([128, dt8, s], BF16, tag="xt", name="xt1")
            nc.gpsimd.dma_start(xt1[:], xtp[:, dt8 : 2 * dt8, :])

            biasc = cpool.tile([128, 1], F32, tag="biasc")
            nc.vector.memset(biasc[:], EXP_BIAS)

            wo_sb = cpool.tile([128, dt8, D], BF16, tag="wo")

            # ---------- pipeline pieces ----------
            def emit_proj(wsb, dst, ch, xt_sb, rope):
                cols = slice(ch * chunk, (ch + 1) * chunk)
                ps = psC.tile([128, chunk], F32, tag="tp", name="proj_ps")
                for dt in range(dt8):
                    nc.tensor.matmul(
                        ps[:],
                        wsb[:, dt, :],
                        xt_sb[:, dt, cols],
                        start=(dt == 0),
                        stop=(dt == dt8 - 1),
                    )
                if not rope:
                    nc.vector.tensor_copy(dst[:, cols], ps[:])
                    return
                tsb = ropepool.tile([128, chunk], BF16, tag="tsb")
                nc.scalar.copy(tsb[:], ps[:])
                rps = psC.tile([128, chunk], F32, tag="tp")
                nc.tensor.matmul(rps[:], mp_sb[:], tsb[:], start=True, stop=True)
                m1 = ropepool.tile([128, chunk], BF16, tag="m1")
                nc.vector.tensor_tensor(
                    m1[:], tsb[:], cs128[:, cols], mybir.AluOpType.mult
                )
                m2 = ropepool.tile([128, chunk], BF16, tag="m2")
                nc.vector.tensor_tensor(
                    m2[:], rps[:], sn128[:, cols], mybir.AluOpType.mult
                )
                nc.vector.tensor_tensor(
                    dst[:, cols], m1[:], m2[:], mybir.AluOpType.add
                )

            def emit_vt_group(ch, vt_sb, v_sb):
                vps = psC.tile([128, 4, 128], BF16, tag="tp")
                for j in range(4):
                    ktt = ch * 4 + j
                    nc.tensor.transpose(
                        vps[:, j, :],
                        vt_sb[:, ktt * 128 : (ktt + 1) * 128],
                        id_sb[:],
                    )
                nc.vector.tensor_copy(
                    v_sb[:, ch * 4 : (ch + 1) * 4, :, 0:HD],
                    vps[:].rearrange("p t (h d) -> p t h d", h=HL),
                )

            def emit_attn_chunk(bi, ch, q_rope, k_rope, v_sb, aohs):
                cols = slice(ch * chunk, (ch + 1) * chunk)
                pts = {}
                for gi, (k0, glen) in enumerate(groups):
                    for h in range(HL):
                        rows = slice(h * HD, (h + 1) * HD)
                        sg = psA.tile([128, GMAX, chunk], F32, tag=f"sc{h}")
                        for j in range(glen):
                            ktt = k0 + j
                            nc.tensor.matmul(
                                sg[:, j, :],
                                k_rope[rows, ktt * 128 : (ktt + 1) * 128],
                                q_rope[rows, cols],
                                start=True,
                                stop=True,
                            )
                        pt = ptpool.tile([128, GMAX, chunk], BF16, tag="pt")
                        nc.scalar.activation(
                            pt[:, :glen, :],
                            sg[:, :glen, :],
                            mybir.ActivationFunctionType.Exp,
                            bias=biasc[:],
                            scale=EXP_SCALE,
                        )
                        pts[(gi, h)] = pt
                for h in range(HL):
                    pv = psB.tile([HD + 1, chunk], F32, tag="pv")
                    for gi, (k0, glen) in enumerate(groups):
                        pt = pts[(gi, h)]
                        for j in range(glen):
                            ktt = k0 + j
                            nc.tensor.matmul(
                                pv[:],
                                v_sb[:, ktt, h, :],
                                pt[:, j, :],
                                start=(ktt == 0),
                                stop=(ktt == kt - 1),
                            )
                    # unnormalized numerator + sigma row; 1/sigma applied
                    # once, consumer-side after the A2A
                    nc.vector.tensor_copy(aohs[h][:, cols], pv[:])

            # ---------- batch-0 QKV ----------
            q0 = qkvpool.tile([DL, s], BF16, tag="q_rope", bufs=1)
            k0_ = qkvpool.tile([DL, s], BF16, tag="k_rope")
            vt0 = qkvpool.tile([DL, s], BF16, tag="vt", bufs=1)
            v0 = qkvpool.tile([128, kt, HL, HD + 1], BF16, tag="v_sb")
            nc.vector.memset(v0[:, :, :, HD : HD + 1], 1.0)
            for ch in range(nch):
                emit_proj(wk_sb, k0_, ch, xt0, rope=True)
                emit_proj(wv_sb, vt0, ch, xt0, rope=False)
                emit_vt_group(ch, vt0, v0)
            for ch in range(nch):
                emit_proj(wq_sb, q0, ch, xt0, rope=True)

            # Wo load: off the critical path, overlaps batch-0 attention
            nc.sync.dma_start(wo_sb[:], wop[:])

            # ---------- batch-0 attention, batch-1 kv interleaved ----------
            ao0 = [
                attpool.tile([HD + 1, s], BF16, tag=f"aoh{h}", name=f"ao0_{h}")
                for h in range(HL)
            ]
            q1 = qkvpool.tile([DL, s], BF16, tag="q_rope", bufs=1)
            k1 = qkvpool.tile([DL, s], BF16, tag="k_rope")
            vt1 = qkvpool.tile([DL, s], BF16, tag="vt", bufs=1)
            v1 = qkvpool.tile([128, kt, HL, HD + 1], BF16, tag="v_sb")
            for ch in range(nch):
                emit_attn_chunk(0, ch, q0, k0_, v0, ao0)
                if ch == 0:
                    nc.vector.memset(v1[:, :, :, HD : HD + 1], 1.0)
                emit_proj(wk_sb, k1, ch, xt1, rope=True)
                emit_proj(wv_sb, vt1, ch, xt1, rope=False)
                emit_vt_group(ch, vt1, v1)

            # ---------- A2A / Wo ----------
            def emit_a2a(aohs, col0, w, tag):
                """AllToAll of tokens [col0, col0 + 8*w) (w per peer).
                rows 0..127: attn dims (h0, h1); rows 128..129: sigma."""
                a2a_in = dram.tile(
                    [N_CORES, DL + HL, w], BF16, tag=f"a2a_in{tag}",
                    name=f"a2a_in{tag}",
                )
                a2a_out = dram.tile(
                    [N_CORES, DL + HL, w], BF16, tag=f"a2a_out{tag}",
                    name=f"a2a_out{tag}",
                )
                for h in range(HL):
                    nc.sync.dma_start(
                        a2a_in[:, h * HD : (h + 1) * HD, :].rearrange(
                            "j r c -> r j c"
                        ),
                        aohs[h][0:HD, col0 : col0 + N_CORES * w].rearrange(
                            "r (j c) -> r j c", j=N_CORES
                        ),
                    )
                    nc.sync.dma_start(
                        a2a_in[:, DL + h : DL + h + 1, :].rearrange("j r c -> r j c"),
                        aohs[h][HD : HD + 1, col0 : col0 + N_CORES * w].rearrange(
                            "r (j c) -> r j c", j=N_CORES
                        ),
                    )
                nc.gpsimd.collective_compute(
                    "AllToAll",
                    mybir.AluOpType.bypass,
                    replica_groups=[list(range(N_CORES))],
                    ins=[a2a_in.opt()],
                    outs=[a2a_out.opt()],
                )
                return a2a_out

            def emit_wo(a2a_out, w, out_row0, tg):
                recv = rcvpool.tile(
                    [DL, N_CORES, w], BF16, tag=f"recv{tg}", name="recv"
                )
                nc.sync.dma_start(
                    recv[:], a2a_out[:, 0:DL, :].rearrange("j r c -> r j c")
                )
                # sigr row h*8+i = sigma of source core i's local head h
                sigr = rcvpool.tile([H, w], BF16, tag=f"sigr{tg}", name="sigr")
                for h in range(HL):
                    nc.sync.dma_start(
                        sigr[h * N_CORES : (h + 1) * N_CORES, :],
                        a2a_out[:, DL + h, :],
                    )
                sigf = nrmpool.tile([H, w], F32, tag=f"sigf{tg}", name="sigf")
                nc.vector.tensor_copy(sigf[:], sigr[:])
                rcpf = nrmpool.tile([H, w], F32, tag=f"rcpf{tg}", name="rcpf")
                nc.vector.reciprocal(rcpf[:], sigf[:])
                rcpb = nrmpool.tile([H, w], BF16, tag=f"rcpb{tg}", name="rcpb")
                nc.vector.tensor_copy(rcpb[:], rcpf[:])
                bcs = rcvpool.tile(
                    [DL, N_CORES, w], BF16, tag=f"bcs{tg}", name="bcs"
                )
                for i2 in range(N_CORES // 2):
                    bcp = psC.tile([128, 2, w], F32, tag="tp", name="bcp")
                    for k in range(2):
                        i = 2 * i2 + k
                        nc.tensor.matmul(
                            bcp[:, k, :],
                            sel_sb[:, i, :],
                            rcpb[:],
                            start=True,
                            stop=True,
                        )
                    nc.vector.tensor_copy(bcs[:, 2 * i2 : 2 * i2 + 2, :], bcp[:])
                nc.vector.tensor_tensor(
                    recv[:], recv[:], bcs[:], mybir.AluOpType.mult
                )
                for j in range(w // 128):
                    osb = outpool.tile([128, D], F32, tag="osb", name="osb")
                    for nco in range(D // chunk):
                        wps = psA.tile(
                            [128, chunk], F32, tag=f"sc{(j + nco) % 2}", name="wps"
                        )
                        for i in range(N_CORES):
                            nc.tensor.matmul(
                                wps[:],
                                recv[:, i, j * 128 : (j + 1) * 128],
                                wo_sb[:, i, nco * chunk : (nco + 1) * chunk],
                                start=(i == 0),
                                stop=(i == N_CORES - 1),
                            )
                        nc.scalar.copy(osb[:, nco * chunk : (nco + 1) * chunk], wps[:])
                    nc.sync.dma_start(
                        out[out_row0 + j * 128 : out_row0 + (j + 1) * 128, :],
                        osb[:],
                    )

            if debug:
                for name, tl in (("dbg_q", q0), ("dbg_k", k0_), ("dbg_v", vt0)):
                    for cch in range(nch):
                        df = outpool.tile([DL, chunk], F32, tag="dbgf")
                        nc.vector.tensor_copy(
                            df[:], tl[:, cch * chunk : (cch + 1) * chunk]
                        )
                        nc.sync.dma_start(
                            {"dbg_q": dbg_q, "dbg_k": dbg_k, "dbg_v": dbg_v}[name][0][
                                :, cch * chunk : (cch + 1) * chunk
                            ],
                            df[:],
                        )

            a2a_out0 = emit_a2a(ao0, 0, shard_half, "b0")

            # ---------- batch-1 q + attention ----------
            ao1 = [
                attpool.tile([HD + 1, s], BF16, tag=f"aoh{h}", name=f"ao1_{h}")
                for h in range(HL)
            ]
            a2a_out1a = None
            emit_proj(wq_sb, q1, 0, xt1, rope=True)
            emit_proj(wq_sb, q1, 1, xt1, rope=True)
            for ch in range(nch):
                if ch + 2 < nch:
                    emit_proj(wq_sb, q1, ch + 2, xt1, rope=True)
                emit_attn_chunk(1, ch, q1, k1, v1, ao1)
                if ch == 1:
                    # first half of batch 1 ships now; hides under ch2-3
                    a2a_out1a = emit_a2a(ao1, 0, shard_half // 2, "b1a")

            if debug:
                for bi, ao in ((0, ao0), (1, ao1)):
                    for h in range(HL):
                        for cch in range(nch):
                            df = outpool.tile([HD, chunk], F32, tag="dbgf2")
                            nc.vector.tensor_copy(
                                df[:], ao[h][0:HD, cch * chunk : (cch + 1) * chunk]
                            )
                            nc.sync.dma_start(
                                dbg_att[
                                    bi,
                                    h * HD : (h + 1) * HD,
                                    cch * chunk : (cch + 1) * chunk,
                                ],
                                df[:],
                            )

            # last collective first, then ALL Wo work fills its latency.
            # tile_wait_until keeps the scheduler from hoisting the Wo chains
            # ahead of batch-1 attention (their collective deps would stall
            # every engine mid-stream).
            a2a_out1b = emit_a2a(ao1, s // 2, shard_half // 2, "b1b")
            with tc.tile_wait_until(1.0):
                emit_wo(a2a_out0, shard_half, 0, "b0")
            with tc.tile_wait_until(1.01):
                emit_wo(a2a_out1a, shard_half // 2, shard_half, "b1a")
            with tc.tile_wait_until(1.02):
                emit_wo(
                    a2a_out1b, shard_half // 2, shard_half + shard_half // 2, "b1b"
                )

    split_excess_waits(nc)
    return nc


def _host_prep(x, cos, sin, b, s):
    """Device-ready layouts shared across cores."""
    bt = b * s
    # x^T in the projection's contraction layout: [128, b*dt8, s]
    xt = np.ascontiguousarray(x.reshape(bt, D).T.astype(BF16_NP))  # [D, b*s]
    xt = (
        xt.reshape(D // 128, 128, b, s)
        .transpose(1, 2, 0, 3)
        .reshape(128, b * (D // 128), s)
    )
    xt = np.ascontiguousarray(xt)
    # doubled, transposed rope tables [128, s]: row p = table[t, p % 32]
    csn = np.ascontiguousarray(np.tile(cos.T, (4, 1)).astype(BF16_NP))
    snn = np.ascontiguousarray(np.tile(sin.T, (4, 1)).astype(BF16_NP))
    # selector for the consumer-side 1/sigma broadcast (sigr is h-major)
    selm = np.zeros((H, N_CORES, 128), dtype=np.float32)
    for i in range(N_CORES):
        for p in range(128):
            selm[(p // HD) * N_CORES + i, i, p] = 1.0
    selb = np.ascontiguousarray(selm.astype(BF16_NP))
    mperm = np.ascontiguousarray(_perm_matrix().astype(BF16_NP))
    ident = np.ascontiguousarray(np.eye(128, dtype=np.float32).astype(BF16_NP))
    return xt, csn, snn, selb, mperm, ident


def _swz(w):  # [D, M] -> [128, dt8, M] bf16
    m = w.shape[1]
    return np.ascontiguousarray(
        np.asarray(w, dtype=np.float32)
        .reshape(D // 128, 128, m)
        .transpose(1, 0, 2)
        .astype(BF16_NP)
    )


def make_in_maps(x, cos, sin, Wq, Wk, Wv, Wo, b, s):
    xt, csn, snn, selb, mperm, ident = _host_prep(
        np.asarray(x, dtype=np.float32),
        np.asarray(cos, dtype=np.float32),
        np.asarray(sin, dtype=np.float32),
        b, s,
    )
    wo_s = _swz(Wo)
    in_maps = []
    for c in range(N_CORES):
        cs = slice(c * DL, (c + 1) * DL)
        in_maps.append(
            {
                "xt": xt,
                "csn": csn,
                "snn": snn,
                "wq": _swz(Wq[:, cs]),
                "wk": _swz(Wk[:, cs]),
                "wv": _swz(Wv[:, cs]),
                "wo": wo_s,
                "sel": selb,
                "mperm": mperm,
                "ident": ident,
            }
        )
    return in_maps


_NC_CACHE = {}


def run(x, cos, sin, Wq, Wk, Wv, Wo, trace=False, chunk=512, pt_bufs=10,
        debug=False):
    b, s, _ = x.shape
    key = (b, s, chunk, pt_bufs, debug)
    if key not in _NC_CACHE:
        _NC_CACHE[key] = build_nc(
            b=b, s=s, chunk=chunk, pt_bufs=pt_bufs, debug=debug
        )
    nc = _NC_CACHE[key]
    in_maps = make_in_maps(x, cos, sin, Wq, Wk, Wv, Wo, b, s)
    res = run_bass_kernel_spmd(nc, in_maps, list(range(N_CORES)), trace=trace)
    sh = s // N_CORES  # 256
    hh = sh // 2  # 128
    b0 = np.concatenate(
        [res.results[c]["out"][0:sh] for c in range(N_CORES)], axis=0
    )
    b1a = np.concatenate(
        [res.results[c]["out"][sh : sh + hh] for c in range(N_CORES)], axis=0
    )
    b1b = np.concatenate(
        [res.results[c]["out"][sh + hh : 2 * sh] for c in range(N_CORES)], axis=0
    )
    full = np.stack([b0, np.concatenate([b1a, b1b], axis=0)], axis=0)
    return full.reshape(b, s, D), res


def kernel(x, cos, sin, Wq, Wk, Wv, Wo):
    out, _ = run(
        np.asarray(x), np.asarray(cos), np.asarray(sin),
        np.asarray(Wq), np.asarray(Wk), np.asarray(Wv), np.asarray(Wo),
    )
    return out.astype(np.float32)


# revision 40
# speedup vs baseline: 1.4068x; 1.0478x over previous
"""Multi-head attention with RoPE on 8 Trainium2 NeuronCores (v3).

Problem: x[2,2048,1024] -> MHA(16 heads, hd=64, NeoX RoPE, non-causal) -> out.

Sharding: tensor-parallel over heads. Each core owns 2 heads. All input
layout work (x^T, bf16 casts, doubled cos/sin tables, weight swizzles,
per-core weight column slices) happens host-side in make_in_maps; the
device kernel is pure compute:

  - q^T,k^T (RoPE'd via a permutation matmul) and v^T projections from the
    pre-transposed x^T, full sequence per core,
  - flash-style attention with *transposed* scores [s_k, s_q]; the softmax
    denominator comes from a fused ones-column in V (constant bias inside
    the exp keeps fp32 range safe),
  - AllToAll redistributes unnormalized numerator + sigma rows, split in
    three (batch 0 | batch 1 first half | batch 1 second half) so only the
    last small collective is exposed; Wo matmuls fill its latency,
  - consumer-side 1/sigma via one reciprocal + selector-matmul broadcast,
  - local Wo matmul produces [256 b0 | 128+128 b1] token rows per core.

All matmuls run in bf16 (fp32 PSUM accumulation); rel-err tolerance 2e-2.
"""

import sys

sys.path.insert(0, "/opt/trn_rl_repo")

import numpy as np  # noqa: E402
import ml_dtypes  # noqa: E402

import concourse.bass as bass  # noqa: E402
import concourse.mybir as mybir  # noqa: E402
import concourse.tile as tile  # noqa: E402
from concourse.bass_utils import run_bass_kernel_spmd  # noqa: E402

N_CORES = 8
D = 1024
H = 16
HD = 64
HL = H // N_CORES  # local heads per core
DL = HL * HD  # 128 local attn dims
EXP_SCALE = 0.125  # 1/sqrt(hd)
EXP_BIAS = -24.0  # exp(s/8 - 24): cancels in softmax, keeps fp32 range safe
GMAX = 2  # score-psum kt-tiles per exp instruction

F32 = mybir.dt.float32
BF16 = mybir.dt.bfloat16
BF16_NP = ml_dtypes.bfloat16


def _kt_groups(kt):
    groups = []
    k0 = 0
    while k0 < kt:
        g = min(GMAX, kt - k0)
        if (kt - k0) % GMAX == 1 and GMAX > 1:
            g = min(GMAX - 1, kt - k0)
        groups.append((k0, g))
        k0 += g
    return groups


def _perm_matrix():
    """lhsT for the rotate_half matmul: qrot^T = lhsT.T @ q^T."""
    mt = np.zeros((DL, DL), dtype=np.float32)
    for o in (0, HD):
        for r in range(HD // 2):
            mt[o + r, o + r + HD // 2] = -1.0
            mt[o + r + HD // 2, o + r] = 1.0
    return np.ascontiguousarray(mt.T)


def split_excess_waits(nc, max_waits=1):
    """This container's walrus rejects >1 semaphore wait per instruction;
    split excess waits onto NoOp carriers on the same engine."""
    for bb in nc.m.functions[0].blocks:
        insts = bb.instructions
        idx = 0
        while idx < len(insts):
            ins = insts[idx]
            si = ins.sync_info
            if si is not None and si.on_wait and len(si.on_wait) > max_waits:
                ow = list(si.on_wait)
                si.on_wait = ow[-max_waits:]
                extra = ow[:-max_waits]
                k = 0
                while extra:
                    chunk, extra = extra[:max_waits], extra[max_waits:]
                    c = mybir.InstNoOp(name=f"{ins.name}-ws{k}", ins=[], outs=[])
                    c.engine = ins.engine
                    c.sync_info = mybir.SyncInfo(on_wait=chunk, on_update=[])
                    nc.register_instruction(c)
                    insts.insert(idx, c)
                    idx += 1
                    k += 1
            idx += 1


def build_nc(b=2, s=2048, chunk=512, pt_bufs=10, debug=False):
    kt = s // 128
    nch = s // chunk
    dt8 = D // 128
    shard_half = s // N_CORES  # 256 tokens per core per batch
    groups = _kt_groups(kt)

    nc = bass.Bass()
    # all layout prep is host-side; everything below is bf16 device-ready
    xtp = nc.declare_dram_parameter("xt", [128, b * dt8, s], BF16, isOutput=False)
    csp = nc.declare_dram_parameter("csn", [128, s], BF16, isOutput=False)
    snp = nc.declare_dram_parameter("snn", [128, s], BF16, isOutput=False)
    wqp = nc.declare_dram_parameter("wq", [128, dt8, DL], BF16, isOutput=False)
    wkp = nc.declare_dram_parameter("wk", [128, dt8, DL], BF16, isOutput=False)
    wvp = nc.declare_dram_parameter("wv", [128, dt8, DL], BF16, isOutput=False)
    wop = nc.declare_dram_parameter("wo", [128, dt8, D], BF16, isOutput=False)
    selp = nc.declare_dram_parameter("sel", [H, N_CORES, 128], BF16, isOutput=False)
    mpp = nc.declare_dram_parameter("mperm", [DL, DL], BF16, isOutput=False)
    idp = nc.declare_dram_parameter("ident", [128, 128], BF16, isOutput=False)
    out = nc.declare_dram_parameter("out", [b * shard_half, D], F32, isOutput=True)
    if debug:
        dbg_q = nc.declare_dram_parameter("dbg_q", [b, DL, s], F32, isOutput=True)
        dbg_k = nc.declare_dram_parameter("dbg_k", [b, DL, s], F32, isOutput=True)
        dbg_v = nc.declare_dram_parameter("dbg_v", [b, DL, s], F32, isOutput=True)
        dbg_att = nc.declare_dram_parameter("dbg_att", [b, DL, s], F32, isOutput=True)

    with tile.TileContext(nc) as tc:
        with (
            tc.tile_pool(name="dram", bufs=1, space="DRAM") as dram,
            tc.tile_pool(name="const", bufs=1) as cpool,
            tc.tile_pool(name="xt", bufs=2) as xtpool,
            tc.tile_pool(name="qkv", bufs=2) as qkvpool,
            tc.tile_pool(name="rope", bufs=2) as ropepool,
            tc.tile_pool(name="pt", bufs=pt_bufs) as ptpool,
            tc.tile_pool(name="att", bufs=2) as attpool,
            tc.tile_pool(name="nrm", bufs=1) as nrmpool,
            tc.tile_pool(name="recv", bufs=1) as rcvpool,
            tc.tile_pool(name="outp", bufs=1) as outpool,
            # PSUM: 8 banks. psA = scores (2 tags x 2 banks; projections and
            # Wo borrow). psB = 2 PV banks. psC = 2 banks for v-transposes /
            # rot / bc broadcasts.
            tc.tile_pool(name="psA", bufs=1, space="PSUM") as psA,
            tc.tile_pool(name="psB", bufs=2, space="PSUM") as psB,
            tc.tile_pool(name="psC", bufs=2, space="PSUM") as psC,
        ):
            # ---------- constants (direct bf16 loads, no staging) ----------
            id_sb = cpool.tile([128, 128], BF16, tag="ident")
            nc.sync.dma_start(id_sb[:], idp[:])
            mp_sb = cpool.tile([DL, DL], BF16, tag="mperm")
            nc.sync.dma_start(mp_sb[:], mpp[:])

            # x^T for both batches (one big DMA each; batch 1's overlaps
            # batch-0 compute)
            xt0 = xtpool.tile([128, dt8, s], BF16, tag="xt", name="xt0")
            nc.sync.dma_start(xt0[:], xtp[:, 0:dt8, :])

            wq_sb = cpool.tile([128, dt8, DL], BF16, tag="wq")
            nc.sync.dma_start(wq_sb[:], wqp[:])
            wk_sb = cpool.tile([128, dt8, DL], BF16, tag="wk")
            nc.sync.dma_start(wk_sb[:], wkp[:])
            wv_sb = cpool.tile([128, dt8, DL], BF16, tag="wv")
            nc.sync.dma_start(wv_sb[:], wvp[:])
            cs128 = cpool.tile([128, s], BF16, tag="cs")
            nc.sync.dma_start(cs128[:], csp[:])
            sn128 = cpool.tile([128, s], BF16, tag="sn")
            nc.sync.dma_start(sn128[:], snp[:])
            sel_sb = cpool.tile([H, N_CORES, 128], BF16, tag="sel")
            nc.sync.dma_start(sel_sb[:], selp[:])

            xt1 = xtpool.tile([128, dt8, s], BF16, tag="xt", name="xt1")
            nc.gpsimd.dma_start(xt1[:], xtp[:, dt8 : 2 * dt8, :])

            biasc = cpool.tile([128, 1], F32, tag="biasc")
            nc.vector.memset(biasc[:], EXP_BIAS)

            wo_sb = cpool.tile([128, dt8, D], BF16, tag="wo")

            # ---------- pipeline pieces ----------
            def emit_proj(wsb, dst, ch, xt_sb, rope):
                cols = slice(ch * chunk, (ch + 1) * chunk)
                ps = psC.tile([128, chunk], F32, tag="tp", name="proj_ps")
                for dt in range(dt8):
                    nc.tensor.matmul(
                        ps[:],
                        wsb[:, dt, :],
                        xt_sb[:, dt, cols],
                        start=(dt == 0),
                        stop=(dt == dt8 - 1),
                    )
                if not rope:
                    nc.vector.tensor_copy(dst[:, cols], ps[:])
                    return
                tsb = ropepool.tile([128, chunk], BF16, tag="tsb")
                nc.scalar.copy(tsb[:], ps[:])
                rps = psC.tile([128, chunk], F32, tag="tp")
                nc.tensor.matmul(rps[:], mp_sb[:], tsb[:], start=True, stop=True)
                m1 = ropepool.tile([128, chunk], BF16, tag="m1")
                nc.vector.tensor_tensor(
                    m1[:], tsb[:], cs128[:, cols], mybir.AluOpType.mult
                )
                m2 = ropepool.tile([128, chunk], BF16, tag="m2")
                nc.vector.tensor_tensor(
                    m2[:], rps[:], sn128[:, cols], mybir.AluOpType.mult
                )
                nc.vector.tensor_tensor(
                    dst[:, cols], m1[:], m2[:], mybir.AluOpType.add
                )

            def emit_vt_group(ch, vt_sb, v_sb):
                vps = psC.tile([128, 4, 128], BF16, tag="tp")
                for j in range(4):
                    ktt = ch * 4 + j
                    nc.tensor.transpose(
                        vps[:, j, :],
                        vt_sb[:, ktt * 128 : (ktt + 1) * 128],
                        id_sb[:],
                    )
                nc.vector.tensor_copy(
                    v_sb[:, ch * 4 : (ch + 1) * 4, :, 0:HD],
                    vps[:].rearrange("p t (h d) -> p t h d", h=HL),
                )

            def emit_attn_chunk(bi, ch, q_rope, k_rope, v_sb, aohs):
                cols = slice(ch * chunk, (ch + 1) * chunk)
                pts = {}
                for gi, (k0, glen) in enumerate(groups):
                    for h in range(HL):
                        rows = slice(h * HD, (h + 1) * HD)
                        sg = psA.tile([128, GMAX, chunk], F32, tag=f"sc{h}")
                        for j in range(glen):
                            ktt = k0 + j
                            nc.tensor.matmul(
                                sg[:, j, :],
                                k_rope[rows, ktt * 128 : (ktt + 1) * 128],
                                q_rope[rows, cols],
                                start=True,
                                stop=True,
                            )
                        pt = ptpool.tile([128, GMAX, chunk], BF16, tag="pt")
                        nc.scalar.activation(
                            pt[:, :glen, :],
                            sg[:, :glen, :],
                            mybir.ActivationFunctionType.Exp,
                            bias=biasc[:],
                            scale=EXP_SCALE,
                        )
                        pts[(gi, h)] = pt
                for h in range(HL):
                    pv = psB.tile([HD + 1, chunk], F32, tag="pv")
                    for gi, (k0, glen) in enumerate(groups):
                        pt = pts[(gi, h)]
                        for j in range(glen):
                            ktt = k0 + j
                            nc.tensor.matmul(
                                pv[:],
                                v_sb[:, ktt, h, :],
                                pt[:, j, :],
                                start=(ktt == 0),
                                stop=(ktt == kt - 1),
                            )
                    # unnormalized numerator + sigma row; 1/sigma applied
                    # once, consumer-side after the A2A
                    nc.vector.tensor_copy(aohs[h][:, cols], pv[:])

            # ---------- batch-0 QKV ----------
            q0 = qkvpool.tile([DL, s], BF16, tag="q_rope", bufs=1)
            k0_ = qkvpool.tile([DL, s], BF16, tag="k_rope")
            vt0 = qkvpool.tile([DL, s], BF16, tag="vt", bufs=1)
            v0 = qkvpool.tile([128, kt, HL, HD + 1], BF16, tag="v_sb")
            nc.vector.memset(v0[:, :, :, HD : HD + 1], 1.0)
            for ch in range(nch):
                emit_proj(wk_sb, k0_, ch, xt0, rope=True)
                emit_proj(wv_sb, vt0, ch, xt0, rope=False)
                emit_vt_group(ch, vt0, v0)
            for ch in range(nch):
                emit_proj(wq_sb, q0, ch, xt0, rope=True)

            # Wo load: off the critical path, overlaps batch-0 attention
            nc.sync.dma_start(wo_sb[:], wop[:])

            # ---------- batch-0 attention, batch-1 kv interleaved ----------
            ao0 = [
                attpool.tile([HD + 1, s], BF16, tag=f"aoh{h}", name=f"ao0_{h}")
                for h in range(HL)
            ]
            q1 = qkvpool.tile([DL, s], BF16, tag="q_rope", bufs=1)
            k1 = qkvpool.tile([DL, s], BF16, tag="k_rope")
            vt1 = qkvpool.tile([DL, s], BF16, tag="vt", bufs=1)
            v1 = qkvpool.tile([128, kt, HL, HD + 1], BF16, tag="v_sb")
            for ch in range(nch):
                emit_attn_chunk(0, ch, q0, k0_, v0, ao0)
                if ch == 0:
                    nc.vector.memset(v1[:, :, :, HD : HD + 1], 1.0)
                emit_proj(wk_sb, k1, ch, xt1, rope=True)
                emit_proj(wv_sb, vt1, ch, xt1, rope=False)
                emit_vt_group(ch, vt1, v1)

            # ---------- A2A / Wo ----------
            def emit_a2a(aohs, col0, w, tag):
                """AllToAll of tokens [col0, col0 + 8*w) (w per peer).
                rows 0..127: attn dims (h0, h1); rows 128..129: sigma."""
                a2a_in = dram.tile(
                    [N_CORES, DL + HL, w], BF16, tag=f"a2a_in{tag}",
                    name=f"a2a_in{tag}",
                )
                a2a_out = dram.tile(
                    [N_CORES, DL + HL, w], BF16, tag=f"a2a_out{tag}",
                    name=f"a2a_out{tag}",
                )
                for h in range(HL):
                    nc.sync.dma_start(
                        a2a_in[:, h * HD : (h + 1) * HD, :].rearrange(
                            "j r c -> r j c"
                        ),
                        aohs[h][0:HD, col0 : col0 + N_CORES * w].rearrange(
                            "r (j c) -> r j c", j=N_CORES
                        ),
                    )
                    nc.sync.dma_start(
                        a2a_in[:, DL + h : DL + h + 1, :].rearrange("j r c -> r j c"),
                        aohs[h][HD : HD + 1, col0 : col0 + N_CORES * w].rearrange(
                            "r (j c) -> r j c", j=N_CORES
                        ),
                    )
                nc.gpsimd.collective_compute(
                    "AllToAll",
                    mybir.AluOpType.bypass,
                    replica_groups=[list(range(N_CORES))],
                    ins=[a2a_in.opt()],
                    outs=[a2a_out.opt()],
                )
                return a2a_out

            def emit_wo(a2a_out, w, out_row0, tg):
                recv = rcvpool.tile(
                    [DL, N_CORES, w], BF16, tag=f"recv{tg}", name="recv"
                )
                nc.sync.dma_start(
                    recv[:], a2a_out[:, 0:DL, :].rearrange("j r c -> r j c")
                )
                # sigr row h*8+i = sigma of source core i's local head h
                sigr = rcvpool.tile([H, w], BF16, tag=f"sigr{tg}", name="sigr")
                for h in range(HL):
                    nc.sync.dma_start(
                        sigr[h * N_CORES : (h + 1) * N_CORES, :],
                        a2a_out[:, DL + h, :],
                    )
                sigf = nrmpool.tile([H, w], F32, tag=f"sigf{tg}", name="sigf")
                nc.vector.tensor_copy(sigf[:], sigr[:])
                rcpf = nrmpool.tile([H, w], F32, tag=f"rcpf{tg}", name="rcpf")
                nc.vector.reciprocal(rcpf[:], sigf[:])
                rcpb = nrmpool.tile([H, w], BF16, tag=f"rcpb{tg}", name="rcpb")
                nc.vector.tensor_copy(rcpb[:], rcpf[:])
                bcs = rcvpool.tile(
                    [DL, N_CORES, w], BF16, tag=f"bcs{tg}", name="bcs"
                )
                for i2 in range(N_CORES // 2):
                    bcp = psC.tile([128, 2, w], F32, tag="tp", name="bcp")
                    for k in range(2):
                        i = 2 * i2 + k
                        nc.tensor.matmul(
                            bcp[:, k, :],
                            sel_sb[:, i, :],
                            rcpb[:],
                            start=True,
                            stop=True,
                        )
                    nc.vector.tensor_copy(bcs[:, 2 * i2 : 2 * i2 + 2, :], bcp[:])
                nc.vector.tensor_tensor(
                    recv[:], recv[:], bcs[:], mybir.AluOpType.mult
                )
                for j in range(w // 128):
                    osb = outpool.tile([128, D], F32, tag="osb", name="osb")
                    for nco in range(D // chunk):
                        wps = psA.tile(
                            [128, chunk], F32, tag=f"sc{(j + nco) % 2}", name="wps"
                        )
                        for i in range(N_CORES):
                            nc.tensor.matmul(
                                wps[:],
                                recv[:, i, j * 128 : (j + 1) * 128],
                                wo_sb[:, i, nco * chunk : (nco + 1) * chunk],
                                start=(i == 0),
                                stop=(i == N_CORES - 1),
                            )
                        nc.scalar.copy(osb[:, nco * chunk : (nco + 1) * chunk], wps[:])
                    nc.sync.dma_start(
                        out[out_row0 + j * 128 : out_row0 + (j + 1) * 128, :],
                        osb[:],
                    )

            if debug:
                for name, tl in (("dbg_q", q0), ("dbg_k", k0_), ("dbg_v", vt0)):
                    for cch in range(nch):
                        df = outpool.tile([DL, chunk], F32, tag="dbgf")
                        nc.vector.tensor_copy(
                            df[:], tl[:, cch * chunk : (cch + 1) * chunk]
                        )
                        nc.sync.dma_start(
                            {"dbg_q": dbg_q, "dbg_k": dbg_k, "dbg_v": dbg_v}[name][0][
                                :, cch * chunk : (cch + 1) * chunk
                            ],
                            df[:],
                        )

            a2a_out0 = emit_a2a(ao0, 0, shard_half, "b0")

            # ---------- batch-1 q + attention ----------
            ao1 = [
                attpool.tile([HD + 1, s], BF16, tag=f"aoh{h}", name=f"ao1_{h}")
                for h in range(HL)
            ]
            a2a_out1a = None
            emit_proj(wq_sb, q1, 0, xt1, rope=True)
            emit_proj(wq_sb, q1, 1, xt1, rope=True)
            for ch in range(nch):
                if ch + 2 < nch:
                    emit_proj(wq_sb, q1, ch + 2, xt1, rope=True)
                emit_attn_chunk(1, ch, q1, k1, v1, ao1)
                if ch == 1:
                    # first half of batch 1 ships now; hides under ch2-3
                    a2a_out1a = emit_a2a(ao1, 0, shard_half // 2, "b1a")

            if debug:
                for bi, ao in ((0, ao0), (1, ao1)):
                    for h in range(HL):
                        for cch in range(nch):
                            df = outpool.tile([HD, chunk], F32, tag="dbgf2")
                            nc.vector.tensor_copy(
                                df[:], ao[h][0:HD, cch * chunk : (cch + 1) * chunk]
                            )
                            nc.sync.dma_start(
                                dbg_att[
                                    bi,
                                    h * HD : (h + 1) * HD,
                                    cch * chunk : (cch + 1) * chunk,
                                ],
                                df[:],
                            )

            # last collective first, then Wo-b0 fills its latency.
            # tile_wait_until keeps the scheduler from hoisting the Wo chains
            # ahead of batch-1 attention (their collective deps would stall
            # every engine mid-stream).
            a2a_out1 = emit_a2a(ao1, 0, shard_half, "b1")
            with tc.tile_wait_until(1.0):
                emit_wo(a2a_out0, shard_half, 0, "b0")
            with tc.tile_wait_until(1.01):
                emit_wo(a2a_out1, shard_half, shard_half, "b1")

    split_excess_waits(nc)
    return nc


def _host_prep(x, cos, sin, b, s):
    """Device-ready layouts shared across cores."""
    bt = b * s
    # x^T in the projection's contraction layout: [128, b*dt8, s]
    xt = np.ascontiguousarray(x.reshape(bt, D).T.astype(BF16_NP))  # [D, b*s]
    xt = (
        xt.reshape(D // 128, 128, b, s)
        .transpose(1, 2, 0, 3)
        .reshape(128, b * (D // 128), s)
    )
    xt = np.ascontiguousarray(xt)
    # doubled, transposed rope tables [128, s]: row p = table[t, p % 32]
    csn = np.ascontiguousarray(np.tile(cos.T, (4, 1)).astype(BF16_NP))
    snn = np.ascontiguousarray(np.tile(sin.T, (4, 1)).astype(BF16_NP))
    # selector for the consumer-side 1/sigma broadcast (sigr is h-major)
    selm = np.zeros((H, N_CORES, 128), dtype=np.float32)
    for i in range(N_CORES):
        for p in range(128):
            selm[(p // HD) * N_CORES + i, i, p] = 1.0
    selb = np.ascontiguousarray(selm.astype(BF16_NP))
    mperm = np.ascontiguousarray(_perm_matrix().astype(BF16_NP))
    ident = np.ascontiguousarray(np.eye(128, dtype=np.float32).astype(BF16_NP))
    return xt, csn, snn, selb, mperm, ident


def _swz(w):  # [D, M] -> [128, dt8, M] bf16
    m = w.shape[1]
    return np.ascontiguousarray(
        np.asarray(w, dtype=np.float32)
        .reshape(D // 128, 128, m)
        .transpose(1, 0, 2)
        .astype(BF16_NP)
    )


def make_in_maps(x, cos, sin, Wq, Wk, Wv, Wo, b, s):
    xt, csn, snn, selb, mperm, ident = _host_prep(
        np.asarray(x, dtype=np.float32),
        np.asarray(cos, dtype=np.float32),
        np.asarray(sin, dtype=np.float32),
        b, s,
    )
    wo_s = _swz(Wo)
    in_maps = []
    for c in range(N_CORES):
        cs = slice(c * DL, (c + 1) * DL)
        in_maps.append(
            {
                "xt": xt,
                "csn": csn,
                "snn": snn,
                "wq": _swz(Wq[:, cs]),
                "wk": _swz(Wk[:, cs]),
                "wv": _swz(Wv[:, cs]),
                "wo": wo_s,
                "sel": selb,
                "mperm": mperm,
                "ident": ident,
            }
        )
    return in_maps


_NC_CACHE = {}


def run(x, cos, sin, Wq, Wk, Wv, Wo, trace=False, chunk=512, pt_bufs=10,
        debug=False):
    b, s, _ = x.shape
    key = (b, s, chunk, pt_bufs, debug)
    if key not in _NC_CACHE:
        _NC_CACHE[key] = build_nc(
            b=b, s=s, chunk=chunk, pt_bufs=pt_bufs, debug=debug
        )
    nc = _NC_CACHE[key]
    in_maps = make_in_maps(x, cos, sin, Wq, Wk, Wv, Wo, b, s)
    res = run_bass_kernel_spmd(nc, in_maps, list(range(N_CORES)), trace=trace)
    sh = s // N_CORES  # 256
    b0 = np.concatenate(
        [res.results[c]["out"][0:sh] for c in range(N_CORES)], axis=0
    )
    b1 = np.concatenate(
        [res.results[c]["out"][sh : 2 * sh] for c in range(N_CORES)], axis=0
    )
    full = np.stack([b0, b1], axis=0)
    return full.reshape(b, s, D), res


def kernel(x, cos, sin, Wq, Wk, Wv, Wo):
    out, _ = run(
        np.asarray(x), np.asarray(cos), np.asarray(sin),
        np.asarray(Wq), np.asarray(Wk), np.asarray(Wv), np.asarray(Wo),
    )
    return out.astype(np.float32)
